# revision 2
# baseline (speedup 1.0000x reference)
"""Multi-head attention (B=2, S=2048, D=1024, H=16) on 8 Trainium2 NeuronCores.

Sharding: core c handles (batch b=c//4, query chunk j=c%4 of 512 rows).
 - QKV projection computed locally for the core's 512 rows (weights replicated,
   pre-transposed + bf16-cast on host; softmax scale folded into W_q).
 - K^T / V are AllGathered across each batch's 4-core group.
 - Attention (all 16 heads, 512 queries x 2048 keys) fully local:
   scoresT = K_h @ Q_h^T  ->  exp on ACT  ->  attnT = [V_h|1]^T @ E
   (ones column gives the softmax denominator Z in row 64 of attnT psum).
 - Output projection local per 512-row chunk; final output assembled on host.
"""

import numpy as np
import ml_dtypes

import concourse.bass as bass
import concourse.mybir as mybir
import concourse.tile as tile
from concourse import bacc
from concourse.bass_utils import run_bass_kernel_spmd

BF16 = mybir.dt.bfloat16
F32 = mybir.dt.float32

B, S, D = 2, 2048, 1024
H, HD = 16, 64
N_CORES = 8
R = 4            # cores per batch group
SL = S // R      # local query rows per core (512)
P = 128
DCH = D // P     # 8 d-chunks
NKK = S // P     # 16 key chunks
ET_Q = D // P    # 8 e-tiles for Q (and for K)
FREE = 512

REPLICA_GROUPS = [[0, 1, 2, 3], [4, 5, 6, 7]]


def build_program():
    nc = bacc.Bacc("TRN2", target_bir_lowering=False, debug=False,
                   num_devices=N_CORES)

    xT = nc.dram_tensor("xT", [D, SL], BF16, kind="ExternalInput")
    wqkvT = nc.dram_tensor("wqkvT", [D, 3 * D], BF16, kind="ExternalInput")
    bqkv = nc.dram_tensor("bqkv", [1, 3 * D], BF16, kind="ExternalInput")
    woutT = nc.dram_tensor("woutT", [D, D], BF16, kind="ExternalInput")
    bout = nc.dram_tensor("bout", [1, D], BF16, kind="ExternalInput")
    out = nc.dram_tensor("out", [SL, D], F32, kind="ExternalOutput")

    with tile.TileContext(nc) as tc:
        _build(nc, tc, xT, wqkvT, bqkv, woutT, bout, out)
    nc.compile()
    return nc


def _build(nc, tc, xT, wqkvT, bqkv, woutT, bout, out):
    from contextlib import ExitStack

    ctx = ExitStack()
    consts = ctx.enter_context(tc.tile_pool(name="consts", bufs=1))
    dram = ctx.enter_context(tc.tile_pool(name="dram", bufs=1, space="DRAM"))

    # ---- constants ----
    ones_bf = consts.tile([1, FREE], BF16, name="ones_bf")
    nc.vector.memset(ones_bf[:], 1.0)
    ones_f32 = consts.tile([1, HD], F32, name="ones_f32")
    nc.vector.memset(ones_f32[:], 1.0)
    bqkv_sb = consts.tile([1, 3 * D], BF16, name="bqkv_sb")
    nc.sync.dma_start(bqkv_sb[:], bqkv.ap())
    bout_sb = consts.tile([1, D], BF16, name="bout_sb")
    nc.sync.dma_start(bout_sb[:], bout.ap())

    # ---- resident input tiles ----
    xt_pool = ctx.enter_context(tc.tile_pool(name="xt", bufs=1))
    xt = []
    for i in range(DCH):
        t = xt_pool.tile([P, SL], BF16, name=f"xt{i}")
        nc.sync.dma_start(t[:], xT.ap()[P * i:P * (i + 1), :])
        xt.append(t)

    # ---- weight stream (whole wqkvT; KV-part first) ----
    w_pool = ctx.enter_context(tc.tile_pool(name="wq", bufs=1))
    EB = 3 * D // FREE  # 6 e-blocks of 512
    wblk = {}
    for eb in (2, 3, 4, 5):          # K then V blocks
        for d in range(DCH):
            t = w_pool.tile([P, FREE], BF16, name=f"w{eb}_{d}")
            nc.gpsimd.dma_start(t[:], wqkvT.ap()[P * d:P * (d + 1),
                                                 FREE * eb:FREE * (eb + 1)])
            wblk[(eb, d)] = t

    # ---- KV shard projection ----
    kshard = []
    vshard = []
    with tc.tile_pool(name="proj_ps", bufs=4, space="PSUM") as proj_ps, \
         tc.tile_pool(name="shard", bufs=1) as shard_pool:
        # K^T shard: out[e, s] for e in [1024, 2048)
        for t in range(ET_Q):
            ps = proj_ps.tile([P, FREE], F32, name=f"psk{t}", tag="proj")
            eb = 2 + t // 4
            co = P * (t % 4)
            for d in range(DCH):
                nc.tensor.matmul(ps[:], wblk[(eb, d)][:, co:co + P], xt[d][:],
                                 start=(d == 0), stop=False)
            nc.tensor.matmul(ps[:], bqkv_sb[:, D + P * t:D + P * (t + 1)],
                             ones_bf[:], start=False, stop=True)
            ksb = shard_pool.tile([P, FREE], BF16, name=f"ksh{t}")
            nc.scalar.copy(ksb[:], ps[:])
            kshard.append(ksb)
        # V shard: out[s, e] natural, e in [2048, 3072)
        for st in range(SL // P):
            vsb = shard_pool.tile([P, D], BF16, name=f"vsh{st}")
            for eb in range(2):
                ps = proj_ps.tile([P, FREE], F32, name=f"psv{st}_{eb}",
                                  tag="proj")
                web = 4 + eb
                for d in range(DCH):
                    nc.tensor.matmul(ps[:], xt[d][:, P * st:P * (st + 1)],
                                     wblk[(web, d)][:], start=(d == 0),
                                     stop=False)
                nc.tensor.matmul(
                    ps[:], ones_bf[:, :P],
                    bqkv_sb[:, 2 * D + FREE * eb:2 * D + FREE * (eb + 1)],
                    start=False, stop=True)
                nc.scalar.copy(vsb[:, FREE * eb:FREE * (eb + 1)], ps[:])
            vshard.append(vsb)

        # ---- collective: AllGather K^T/V shards within batch group ----
        kvin = dram.tile([16, P, FREE], BF16, name="kvin")
        for t in range(ET_Q):
            nc.gpsimd.dma_start(kvin[t], kshard[t][:])
        for st in range(SL // P):
            nc.gpsimd.dma_start(kvin[8 + 2 * st], vshard[st][:, 0:FREE])
            nc.gpsimd.dma_start(kvin[9 + 2 * st], vshard[st][:, FREE:2 * FREE])
        kvout = dram.tile([R, 16, P, FREE], BF16, name="kvout")
        nc.gpsimd.collective_compute(
            "AllGather", mybir.AluOpType.bypass,
            replica_groups=REPLICA_GROUPS,
            ins=[kvin.opt()], outs=[kvout.opt()],
        )

    # ---- Q projection (reuses freed weight slots) ----
    for eb in (0, 1):
        for d in range(DCH):
            t = w_pool.tile([P, FREE], BF16, name=f"w{eb}_{d}")
            nc.gpsimd.dma_start(t[:], wqkvT.ap()[P * d:P * (d + 1),
                                                 FREE * eb:FREE * (eb + 1)])
            wblk[(eb, d)] = t
    qt_pool = ctx.enter_context(tc.tile_pool(name="qt", bufs=1))
    qt = []
    with tc.tile_pool(name="projq_ps", bufs=4, space="PSUM") as projq_ps:
        for t in range(ET_Q):
            ps = projq_ps.tile([P, FREE], F32, name=f"psq{t}", tag="proj")
            eb = t // 4
            co = P * (t % 4)
            for d in range(DCH):
                nc.tensor.matmul(ps[:], wblk[(eb, d)][:, co:co + P], xt[d][:],
                                 start=(d == 0), stop=False)
            nc.tensor.matmul(ps[:], bqkv_sb[:, P * t:P * (t + 1)], ones_bf[:],
                             start=False, stop=True)
            q = qt_pool.tile([P, FREE], BF16, name=f"qt{t}")
            nc.scalar.copy(q[:], ps[:])
            qt.append(q)

    # ---- KV readback into attention layout ----
    kv_pool = ctx.enter_context(tc.tile_pool(name="kv", bufs=1))
    kt = []
    for t in range(ET_Q):
        k = kv_pool.tile([P, S], BF16, name=f"kt{t}")
        for r in range(R):
            nc.gpsimd.dma_start(k[:, FREE * r:FREE * (r + 1)], kvout[r, t])
        kt.append(k)
    vt = []
    for g in range(NKK):
        r, i = g // 4, g % 4
        v = kv_pool.tile([P, H, HD + 1], BF16, name=f"vt{g}")
        nc.gpsimd.dma_start(
            v[:, 0:8, 0:HD],
            kvout[r, 8 + 2 * i].rearrange("p (h d) -> p h d", d=HD))
        nc.gpsimd.dma_start(
            v[:, 8:16, 0:HD],
            kvout[r, 9 + 2 * i].rearrange("p (h d) -> p h d", d=HD))
        nc.vector.memset(v[:, :, HD:HD + 1], 1.0)
        vt.append(v)

    # ---- prefetch output-projection weights ----
    wo_pool = ctx.enter_context(tc.tile_pool(name="wo", bufs=1))
    wo = []
    for p_ in range(DCH):
        t = wo_pool.tile([P, D], BF16, name=f"wo{p_}")
        nc.sync.dma_start(t[:], woutT.ap()[P * p_:P * (p_ + 1), :])
        wo.append(t)

    # ---- attention ----
    attn_sb_pool = ctx.enter_context(tc.tile_pool(name="attnsb", bufs=1))
    small_pool = ctx.enter_context(tc.tile_pool(name="small", bufs=4))
    attn_sb = [attn_sb_pool.tile([P, FREE], BF16, name=f"attnsb{p_}")
               for p_ in range(H // 2)]
    GRP = 2  # kk-chunks per score-psum tile
    with tc.tile_pool(name="sc_ps", bufs=2, space="PSUM") as sc_ps, \
         tc.tile_pool(name="at_ps", bufs=2, space="PSUM") as at_ps, \
         tc.tile_pool(name="bc_ps", bufs=2, space="PSUM") as bc_ps, \
         tc.tile_pool(name="e_sb", bufs=4) as e_pool:
        for h in range(H):
            ktile, koff = h // 2, HD * (h % 2)
            q_rhs = qt[ktile][koff:koff + HD, :]
            at = at_ps.tile([HD + 1, FREE], F32, name=f"at{h}", tag="at")
            for g in range(NKK // GRP):
                sc = sc_ps.tile([P, GRP * FREE], F32, name=f"sc{h}_{g}",
                                tag="sc")
                for j in range(GRP):
                    kk = GRP * g + j
                    nc.tensor.matmul(
                        sc[:, FREE * j:FREE * (j + 1)],
                        kt[ktile][koff:koff + HD, P * kk:P * (kk + 1)],
                        q_rhs, start=True, stop=True)
                e = e_pool.tile([P, GRP * FREE], BF16, name=f"e{h}_{g}",
                                tag="e")
                nc.scalar.activation(e[:], sc[:],
                                     mybir.ActivationFunctionType.Exp)
                for j in range(GRP):
                    kk = GRP * g + j
                    nc.tensor.matmul(at[:], vt[kk][:, h, :],
                                     e[:, FREE * j:FREE * (j + 1)],
                                     start=(kk == 0), stop=(kk == NKK - 1))
            # normalize: attn = attnT[0:64] * (1/Z) ; Z = attnT row 64
            rz = small_pool.tile([1, FREE], F32, name=f"rz{h}", tag="rz")
            nc.vector.reciprocal(rz[:], at[HD:HD + 1, :])
            bc = bc_ps.tile([HD, FREE], F32, name=f"bc{h}", tag="bc")
            nc.tensor.matmul(bc[:], ones_f32[:], rz[:], start=True, stop=True)
            bcs = small_pool.tile([HD, FREE], F32, name=f"bcs{h}", tag="bcs")
            nc.vector.tensor_copy(bcs[:], bc[:])
            nc.vector.tensor_mul(attn_sb[h // 2][koff:koff + HD, :],
                                 at[0:HD, :], bcs[:])

    # ---- output projection ----
    with tc.tile_pool(name="out_ps", bufs=1, space="PSUM") as out_ps, \
         tc.tile_pool(name="out_sb", bufs=2) as out_sb_pool:
        ops = [[out_ps.tile([P, FREE], F32, name=f"op{st}_{eb}")
                for eb in range(2)] for st in range(SL // P)]
        for p_ in range(DCH):
            for st in range(SL // P):
                for eb in range(2):
                    nc.tensor.matmul(ops[st][eb],
                                     attn_sb[p_][:, P * st:P * (st + 1)],
                                     wo[p_][:, FREE * eb:FREE * (eb + 1)],
                                     start=(p_ == 0), stop=False)
        for st in range(SL // P):
            for eb in range(2):
                nc.tensor.matmul(ops[st][eb], ones_bf[:, :P],
                                 bout_sb[:, FREE * eb:FREE * (eb + 1)],
                                 start=False, stop=True)
            osb = out_sb_pool.tile([P, D], F32, name=f"osb{st}", tag="osb")
            for eb in range(2):
                nc.vector.tensor_copy(osb[:, FREE * eb:FREE * (eb + 1)],
                                      ops[st][eb])
            nc.sync.dma_start(out.ap()[P * st:P * (st + 1), :], osb[:])

    ctx.close()


_CACHE = {}


def _get_program():
    if "nc" not in _CACHE:
        _CACHE["nc"] = build_program()
    return _CACHE["nc"]


def prep_inputs(input_tensor, qkv_weight, qkv_bias, out_weight, out_bias):
    """Host-side shard + transpose + cast. Returns in_maps for 8 cores."""
    x = np.asarray(input_tensor, np.float32)
    wqkv = np.asarray(qkv_weight, np.float32).copy()
    bq = np.asarray(qkv_bias, np.float32).copy()
    scale = 1.0 / np.sqrt(np.float32(HD))
    wqkv[:D] *= scale
    bq[:D] *= scale
    bf = ml_dtypes.bfloat16
    wqkvT = np.ascontiguousarray(wqkv.T).astype(bf)
    bqkv = bq.reshape(1, 3 * D).astype(bf)
    woutT = np.ascontiguousarray(np.asarray(out_weight, np.float32).T).astype(bf)
    bout = np.asarray(out_bias, np.float32).reshape(1, D).astype(bf)
    in_maps = []
    for c in range(N_CORES):
        b, j = c // R, c % R
        xTc = np.ascontiguousarray(x[b, SL * j:SL * (j + 1), :].T).astype(bf)
        in_maps.append({"xT": xTc, "wqkvT": wqkvT, "bqkv": bqkv,
                        "woutT": woutT, "bout": bout})
    return in_maps


def kernel(input_tensor, qkv_weight, qkv_bias, out_weight, out_bias,
           **run_kwargs):
    nc = _get_program()
    in_maps = prep_inputs(input_tensor, qkv_weight, qkv_bias, out_weight,
                          out_bias)
    res = run_bass_kernel_spmd(nc, in_maps, core_ids=list(range(N_CORES)),
                               **run_kwargs)
    full = np.empty((B, S, D), np.float32)
    for c in range(N_CORES):
        b, j = c // R, c % R
        full[b, SL * j:SL * (j + 1), :] = res.results[c]["out"]
    if run_kwargs:
        kernel.last_results = res
    return full


# revision 11
# speedup vs baseline: 1.0127x; 1.0127x over previous
"""Multi-head attention (B=2, S=2048, D=1024, H=16) on 8 Trainium2 NeuronCores.

Sharding: core c handles (batch b=c//4, query chunk j=c%4 of 512 rows).
 - QKV projection computed locally for the core's 512 rows (weights replicated,
   pre-transposed + bf16-cast on host; softmax scale folded into W_q).
 - K^T / V are AllGathered across each batch's 4-core group.
 - Attention (all 16 heads, 512 queries x 2048 keys) fully local:
   scoresT = K_h @ Q_h^T  ->  exp on ACT  ->  attnT = [V_h|1]^T @ E
   (ones column gives the softmax denominator Z in row 64 of attnT psum).
 - Output projection local per 512-row chunk; final output assembled on host.
"""

import numpy as np
import ml_dtypes

import concourse.bass as bass
import concourse.mybir as mybir
import concourse.tile as tile
from concourse import bacc
from concourse.bass_utils import run_bass_kernel_spmd

BF16 = mybir.dt.bfloat16
F32 = mybir.dt.float32

B, S, D = 2, 2048, 1024
H, HD = 16, 64
N_CORES = 8
R = 4            # cores per batch group
SL = S // R      # local query rows per core (512)
P = 128
DCH = D // P     # 8 d-chunks
NKK = S // P     # 16 key chunks
ET_Q = D // P    # 8 e-tiles for Q (and for K)
FREE = 512

REPLICA_GROUPS = [[0, 1, 2, 3], [4, 5, 6, 7]]


def build_program():
    nc = bacc.Bacc("TRN2", target_bir_lowering=False, debug=False,
                   num_devices=N_CORES)

    xT = nc.dram_tensor("xT", [D, SL], BF16, kind="ExternalInput")
    wqkvT = nc.dram_tensor("wqkvT", [D, 3 * D], BF16, kind="ExternalInput")
    bqkv = nc.dram_tensor("bqkv", [1, 3 * D], BF16, kind="ExternalInput")
    woutT = nc.dram_tensor("woutT", [D, D], BF16, kind="ExternalInput")
    bout = nc.dram_tensor("bout", [1, D], BF16, kind="ExternalInput")
    out = nc.dram_tensor("out", [SL, D], F32, kind="ExternalOutput")

    with tile.TileContext(nc) as tc:
        _build(nc, tc, xT, wqkvT, bqkv, woutT, bout, out)
    nc.compile()
    return nc


def _build(nc, tc, xT, wqkvT, bqkv, woutT, bout, out):
    from contextlib import ExitStack

    ctx = ExitStack()
    consts = ctx.enter_context(tc.tile_pool(name="consts", bufs=1))
    dram = ctx.enter_context(tc.tile_pool(name="dram", bufs=1, space="DRAM"))

    # ---- constants ----
    ones_bf = consts.tile([1, FREE], BF16, name="ones_bf")
    nc.vector.memset(ones_bf[:], 1.0)
    ones_f32 = consts.tile([1, HD], F32, name="ones_f32")
    nc.vector.memset(ones_f32[:], 1.0)
    bqkv_sb = consts.tile([1, 3 * D], BF16, name="bqkv_sb")
    nc.sync.dma_start(bqkv_sb[:], bqkv.ap())
    bout_sb = consts.tile([1, D], BF16, name="bout_sb")
    nc.sync.dma_start(bout_sb[:], bout.ap())

    # ---- resident input tiles ----
    xt_pool = ctx.enter_context(tc.tile_pool(name="xt", bufs=1))
    xt = []
    for i in range(DCH):
        t = xt_pool.tile([P, SL], BF16, name=f"xt{i}")
        nc.sync.dma_start(t[:], xT.ap()[P * i:P * (i + 1), :])
        xt.append(t)

    # ---- weight stream (whole wqkvT; KV-part first) ----
    w_pool = ctx.enter_context(tc.tile_pool(name="wq", bufs=1))
    EB = 3 * D // FREE  # 6 e-blocks of 512
    wblk = {}
    for eb in (2, 3, 4, 5):          # K then V blocks
        for d in range(DCH):
            t = w_pool.tile([P, FREE], BF16, name=f"w{eb}_{d}")
            nc.gpsimd.dma_start(t[:], wqkvT.ap()[P * d:P * (d + 1),
                                                 FREE * eb:FREE * (eb + 1)])
            wblk[(eb, d)] = t

    # ---- KV shard projection ----
    kshard = []
    vshard = []
    with tc.tile_pool(name="proj_ps", bufs=4, space="PSUM") as proj_ps, \
         tc.tile_pool(name="shard", bufs=1) as shard_pool:
        # K^T shard: out[e, s] for e in [1024, 2048)
        for t in range(ET_Q):
            ps = proj_ps.tile([P, FREE], F32, name=f"psk{t}", tag="proj")
            eb = 2 + t // 4
            co = P * (t % 4)
            for d in range(DCH):
                nc.tensor.matmul(ps[:], wblk[(eb, d)][:, co:co + P], xt[d][:],
                                 start=(d == 0), stop=False)
            nc.tensor.matmul(ps[:], bqkv_sb[:, D + P * t:D + P * (t + 1)],
                             ones_bf[:], start=False, stop=True)
            ksb = shard_pool.tile([P, FREE], BF16, name=f"ksh{t}")
            nc.scalar.copy(ksb[:], ps[:])
            kshard.append(ksb)
        # V shard: out[s, e] natural, e in [2048, 3072)
        for st in range(SL // P):
            vsb = shard_pool.tile([P, D], BF16, name=f"vsh{st}")
            for eb in range(2):
                ps = proj_ps.tile([P, FREE], F32, name=f"psv{st}_{eb}",
                                  tag="proj")
                web = 4 + eb
                for d in range(DCH):
                    nc.tensor.matmul(ps[:], xt[d][:, P * st:P * (st + 1)],
                                     wblk[(web, d)][:], start=(d == 0),
                                     stop=False)
                nc.tensor.matmul(
                    ps[:], ones_bf[:, :P],
                    bqkv_sb[:, 2 * D + FREE * eb:2 * D + FREE * (eb + 1)],
                    start=False, stop=True)
                nc.scalar.copy(vsb[:, FREE * eb:FREE * (eb + 1)], ps[:])
            vshard.append(vsb)

        # ---- collectives: AllGather K^T then V shards within batch group ----
        kin = dram.tile([8, P, FREE], BF16, name="kin")
        for t in range(ET_Q):
            nc.gpsimd.dma_start(kin[t], kshard[t][:])
        kout = dram.tile([R, 8, P, FREE], BF16, name="kout")
        nc.gpsimd.collective_compute(
            "AllGather", mybir.AluOpType.bypass,
            replica_groups=REPLICA_GROUPS,
            ins=[kin.opt()], outs=[kout.opt()],
        )
        vin = dram.tile([8, P, FREE], BF16, name="vin")
        for st in range(SL // P):
            nc.gpsimd.dma_start(vin[2 * st], vshard[st][:, 0:FREE])
            nc.gpsimd.dma_start(vin[2 * st + 1], vshard[st][:, FREE:2 * FREE])
        vout = dram.tile([R, 8, P, FREE], BF16, name="vout")
        nc.gpsimd.collective_compute(
            "AllGather", mybir.AluOpType.bypass,
            replica_groups=REPLICA_GROUPS,
            ins=[vin.opt()], outs=[vout.opt()],
        )

    # ---- Q projection (reuses freed weight slots) ----
    for eb in (0, 1):
        for d in range(DCH):
            t = w_pool.tile([P, FREE], BF16, name=f"w{eb}_{d}")
            nc.gpsimd.dma_start(t[:], wqkvT.ap()[P * d:P * (d + 1),
                                                 FREE * eb:FREE * (eb + 1)])
            wblk[(eb, d)] = t
    qt_pool = ctx.enter_context(tc.tile_pool(name="qt", bufs=1))
    qt = []
    with tc.tile_pool(name="projq_ps", bufs=4, space="PSUM") as projq_ps:
        for t in range(ET_Q):
            ps = projq_ps.tile([P, FREE], F32, name=f"psq{t}", tag="proj")
            eb = t // 4
            co = P * (t % 4)
            for d in range(DCH):
                nc.tensor.matmul(ps[:], wblk[(eb, d)][:, co:co + P], xt[d][:],
                                 start=(d == 0), stop=False)
            nc.tensor.matmul(ps[:], bqkv_sb[:, P * t:P * (t + 1)], ones_bf[:],
                             start=False, stop=True)
            q = qt_pool.tile([P, FREE], BF16, name=f"qt{t}")
            nc.scalar.copy(q[:], ps[:])
            qt.append(q)

    # ---- KV readback into attention layout (one DMA per tile) ----
    kv_pool = ctx.enter_context(tc.tile_pool(name="kv", bufs=1))
    kt = []
    for t in range(ET_Q):
        k = kv_pool.tile([P, S], BF16, name=f"kt{t}")
        nc.sync.dma_start(k.rearrange("p (r c) -> p r c", r=R),
                          kout[:, t].rearrange("r p c -> p r c"))
        kt.append(k)
    vt = []
    for g in range(NKK):
        r, i = g // 4, g % 4
        v = kv_pool.tile([P, H, HD + 1], BF16, name=f"vt{g}")
        for b2 in range(2):
            nc.sync.dma_start(
                v[:, 8 * b2:8 * (b2 + 1), 0:HD],
                vout[r, 2 * i + b2].rearrange("p (h d) -> p h d", d=HD))
        nc.vector.memset(v[:, :, HD:HD + 1], 1.0)
        vt.append(v)

    # ---- prefetch output-projection weights ----
    wo_pool = ctx.enter_context(tc.tile_pool(name="wo", bufs=1))
    wo = []
    for p_ in range(DCH):
        t = wo_pool.tile([P, D], BF16, name=f"wo{p_}")
        nc.sync.dma_start(t[:], woutT.ap()[P * p_:P * (p_ + 1), :])
        wo.append(t)

    # ---- attention ----
    attn_sb_pool = ctx.enter_context(tc.tile_pool(name="attnsb", bufs=1))
    small_pool = ctx.enter_context(tc.tile_pool(name="small", bufs=4))
    attn_sb = [attn_sb_pool.tile([P, FREE], BF16, name=f"attnsb{p_}")
               for p_ in range(H // 2)]
    GRP = 2  # kk-chunks per score-psum tile
    with tc.tile_pool(name="sc_ps", bufs=2, space="PSUM") as sc_ps, \
         tc.tile_pool(name="at_ps", bufs=2, space="PSUM") as at_ps, \
         tc.tile_pool(name="bc_ps", bufs=2, space="PSUM") as bc_ps, \
         tc.tile_pool(name="e_sb", bufs=4) as e_pool:
        pending = []

        def normalize(h, at):
            # attn = attnT[0:64] * (1/Z) ; Z = attnT row 64
            koff = HD * (h % 2)
            rz = small_pool.tile([1, FREE], F32, name=f"rz{h}", tag="rz")
            nc.vector.reciprocal(rz[:], at[HD:HD + 1, :])
            bc = bc_ps.tile([HD, FREE], F32, name=f"bc{h}", tag="bc")
            nc.tensor.matmul(bc[:], ones_f32[:], rz[:], start=True, stop=True)
            rzb = small_pool.tile([HD, FREE], F32, name=f"rzb{h}", tag="rzb")
            nc.vector.tensor_copy(rzb[:], bc[:])
            nc.vector.tensor_mul(attn_sb[h // 2][koff:koff + HD, :],
                                 at[0:HD, :], rzb[:])

        for h in range(H):
            ktile, koff = h // 2, HD * (h % 2)
            q_rhs = qt[ktile][koff:koff + HD, :]
            at = at_ps.tile([HD + 1, FREE], F32, name=f"at{h}", tag="at")
            for g in range(NKK // GRP):
                sc = sc_ps.tile([P, GRP * FREE], F32, name=f"sc{h}_{g}",
                                tag="sc")
                for j in range(GRP):
                    kk = GRP * g + j
                    nc.tensor.matmul(
                        sc[:, FREE * j:FREE * (j + 1)],
                        kt[ktile][koff:koff + HD, P * kk:P * (kk + 1)],
                        q_rhs, start=True, stop=True)
                e = e_pool.tile([P, GRP * FREE], BF16, name=f"e{h}_{g}",
                                tag="e")
                nc.scalar.activation(e[:], sc[:],
                                     mybir.ActivationFunctionType.Exp)
                for j in range(GRP):
                    kk = GRP * g + j
                    nc.tensor.matmul(at[:], vt[kk][:, h, :],
                                     e[:, FREE * j:FREE * (j + 1)],
                                     start=(kk == 0), stop=(kk == NKK - 1))
                if g == 1 and pending:
                    # deferred normalize of the previous head, off the
                    # critical path of this head's PE stream
                    normalize(*pending.pop())
            pending.append((h, at))
        normalize(*pending.pop())

    # ---- output projection ----
    with tc.tile_pool(name="out_ps", bufs=1, space="PSUM") as out_ps, \
         tc.tile_pool(name="out_sb", bufs=2) as out_sb_pool:
        ops = [[out_ps.tile([P, FREE], F32, name=f"op{st}_{eb}")
                for eb in range(2)] for st in range(SL // P)]
        for p_ in range(DCH):
            for st in range(SL // P):
                for eb in range(2):
                    nc.tensor.matmul(ops[st][eb],
                                     attn_sb[p_][:, P * st:P * (st + 1)],
                                     wo[p_][:, FREE * eb:FREE * (eb + 1)],
                                     start=(p_ == 0), stop=False)
        for st in range(SL // P):
            for eb in range(2):
                nc.tensor.matmul(ops[st][eb], ones_bf[:, :P],
                                 bout_sb[:, FREE * eb:FREE * (eb + 1)],
                                 start=False, stop=True)
            osb = out_sb_pool.tile([P, D], F32, name=f"osb{st}", tag="osb")
            for eb in range(2):
                nc.vector.tensor_copy(osb[:, FREE * eb:FREE * (eb + 1)],
                                      ops[st][eb])
            nc.sync.dma_start(out.ap()[P * st:P * (st + 1), :], osb[:])

    ctx.close()


_CACHE = {}


def _get_program():
    if "nc" not in _CACHE:
        _CACHE["nc"] = build_program()
    return _CACHE["nc"]


def prep_inputs(input_tensor, qkv_weight, qkv_bias, out_weight, out_bias):
    """Host-side shard + transpose + cast. Returns in_maps for 8 cores."""
    x = np.asarray(input_tensor, np.float32)
    wqkv = np.asarray(qkv_weight, np.float32).copy()
    bq = np.asarray(qkv_bias, np.float32).copy()
    scale = 1.0 / np.sqrt(np.float32(HD))
    wqkv[:D] *= scale
    bq[:D] *= scale
    bf = ml_dtypes.bfloat16
    wqkvT = np.ascontiguousarray(wqkv.T).astype(bf)
    bqkv = bq.reshape(1, 3 * D).astype(bf)
    woutT = np.ascontiguousarray(np.asarray(out_weight, np.float32).T).astype(bf)
    bout = np.asarray(out_bias, np.float32).reshape(1, D).astype(bf)
    in_maps = []
    for c in range(N_CORES):
        b, j = c // R, c % R
        xTc = np.ascontiguousarray(x[b, SL * j:SL * (j + 1), :].T).astype(bf)
        in_maps.append({"xT": xTc, "wqkvT": wqkvT, "bqkv": bqkv,
                        "woutT": woutT, "bout": bout})
    return in_maps


def kernel(input_tensor, qkv_weight, qkv_bias, out_weight, out_bias,
           **run_kwargs):
    nc = _get_program()
    in_maps = prep_inputs(input_tensor, qkv_weight, qkv_bias, out_weight,
                          out_bias)
    res = run_bass_kernel_spmd(nc, in_maps, core_ids=list(range(N_CORES)),
                               **run_kwargs)
    full = np.empty((B, S, D), np.float32)
    for c in range(N_CORES):
        b, j = c // R, c % R
        full[b, SL * j:SL * (j + 1), :] = res.results[c]["out"]
    if run_kwargs:
        kernel.last_results = res
    return full


# revision 14
# speedup vs baseline: 1.1056x; 1.0917x over previous
"""Multi-head attention (B=2, S=2048, D=1024, H=16) on 8 Trainium2 NeuronCores.

Sharding: core c handles (batch b=c//4, query chunk j=c%4 of 512 rows).
 - QKV projection computed locally for the core's 512 rows (weights replicated,
   pre-transposed + bf16-cast on host; softmax scale folded into W_q).
 - K^T / V are AllGathered across each batch's 4-core group.
 - Attention (all 16 heads, 512 queries x 2048 keys) fully local:
   scoresT = K_h @ Q_h^T  ->  exp on ACT  ->  attnT = [V_h|1]^T @ E
   (ones column gives the softmax denominator Z in row 64 of attnT psum).
 - Output projection local per 512-row chunk; final output assembled on host.
"""

import numpy as np
import ml_dtypes

import concourse.bass as bass
import concourse.mybir as mybir
import concourse.tile as tile
from concourse import bacc
from concourse.bass_utils import run_bass_kernel_spmd

BF16 = mybir.dt.bfloat16
F32 = mybir.dt.float32

B, S, D = 2, 2048, 1024
H, HD = 16, 64
N_CORES = 8
R = 4            # cores per batch group
SL = S // R      # local query rows per core (512)
P = 128
DCH = D // P     # 8 d-chunks
NKK = S // P     # 16 key chunks
ET_Q = D // P    # 8 e-tiles for Q (and for K)
FREE = 512

REPLICA_GROUPS = [[0, 1, 2, 3], [4, 5, 6, 7]]


def build_program():
    nc = bacc.Bacc("TRN2", target_bir_lowering=False, debug=False,
                   num_devices=N_CORES)

    xT = nc.dram_tensor("xT", [D, SL], BF16, kind="ExternalInput")
    wqkvT = nc.dram_tensor("wqkvT", [D, 3 * D], BF16, kind="ExternalInput")
    bqkv = nc.dram_tensor("bqkv", [1, 3 * D], BF16, kind="ExternalInput")
    woutT = nc.dram_tensor("woutT", [D, D], BF16, kind="ExternalInput")
    bout = nc.dram_tensor("bout", [1, D], BF16, kind="ExternalInput")
    out = nc.dram_tensor("out", [SL, D], F32, kind="ExternalOutput")

    with tile.TileContext(nc) as tc:
        _build(nc, tc, xT, wqkvT, bqkv, woutT, bout, out)
    nc.compile()
    return nc


def _build(nc, tc, xT, wqkvT, bqkv, woutT, bout, out):
    from contextlib import ExitStack

    ctx = ExitStack()
    consts = ctx.enter_context(tc.tile_pool(name="consts", bufs=1))
    dram = ctx.enter_context(tc.tile_pool(name="dram", bufs=1, space="DRAM"))

    # ---- constants ----
    ones_bf = consts.tile([1, FREE], BF16, name="ones_bf")
    nc.vector.memset(ones_bf[:], 1.0)
    bqkv_sb = consts.tile([1, 3 * D], BF16, name="bqkv_sb")
    nc.sync.dma_start(bqkv_sb[:], bqkv.ap())
    bout_sb = consts.tile([1, D], BF16, name="bout_sb")
    nc.sync.dma_start(bout_sb[:], bout.ap())

    # ---- resident input tiles ----
    xt_pool = ctx.enter_context(tc.tile_pool(name="xt", bufs=1))
    xt = []
    for i in range(DCH):
        t = xt_pool.tile([P, SL], BF16, name=f"xt{i}")
        nc.sync.dma_start(t[:], xT.ap()[P * i:P * (i + 1), :])
        xt.append(t)

    # ---- weight stream (whole wqkvT; KV-part first) ----
    w_pool = ctx.enter_context(tc.tile_pool(name="wq", bufs=1))
    EB = 3 * D // FREE  # 6 e-blocks of 512
    wblk = {}
    for eb in (2, 3, 4, 5):          # K then V blocks
        for d in range(DCH):
            t = w_pool.tile([P, FREE], BF16, name=f"w{eb}_{d}")
            nc.gpsimd.dma_start(t[:], wqkvT.ap()[P * d:P * (d + 1),
                                                 FREE * eb:FREE * (eb + 1)])
            wblk[(eb, d)] = t

    # ---- KV shard projection ----
    kshard = []
    vshard = []
    with tc.tile_pool(name="proj_ps", bufs=4, space="PSUM") as proj_ps, \
         tc.tile_pool(name="shard", bufs=1) as shard_pool:
        # K^T shard: out[e, s] for e in [1024, 2048)
        for t in range(ET_Q):
            ps = proj_ps.tile([P, FREE], F32, name=f"psk{t}", tag="proj")
            eb = 2 + t // 4
            co = P * (t % 4)
            for d in range(DCH):
                nc.tensor.matmul(ps[:], wblk[(eb, d)][:, co:co + P], xt[d][:],
                                 start=(d == 0), stop=False)
            nc.tensor.matmul(ps[:], bqkv_sb[:, D + P * t:D + P * (t + 1)],
                             ones_bf[:], start=False, stop=True)
            ksb = shard_pool.tile([P, FREE], BF16, name=f"ksh{t}")
            nc.scalar.copy(ksb[:], ps[:])
            kshard.append(ksb)
        # V shard: out[s, e] natural, e in [2048, 3072)
        for st in range(SL // P):
            vsb = shard_pool.tile([P, D], BF16, name=f"vsh{st}")
            for eb in range(2):
                ps = proj_ps.tile([P, FREE], F32, name=f"psv{st}_{eb}",
                                  tag="proj")
                web = 4 + eb
                for d in range(DCH):
                    nc.tensor.matmul(ps[:], xt[d][:, P * st:P * (st + 1)],
                                     wblk[(web, d)][:], start=(d == 0),
                                     stop=False)
                nc.tensor.matmul(
                    ps[:], ones_bf[:, :P],
                    bqkv_sb[:, 2 * D + FREE * eb:2 * D + FREE * (eb + 1)],
                    start=False, stop=True)
                nc.scalar.copy(vsb[:, FREE * eb:FREE * (eb + 1)], ps[:])
            vshard.append(vsb)

        # ---- collectives: AllGather K^T then V shards within batch group ----
        kin = dram.tile([8, P, FREE], BF16, name="kin")
        for t in range(ET_Q):
            nc.gpsimd.dma_start(kin[t], kshard[t][:])
        kout = dram.tile([R, 8, P, FREE], BF16, name="kout")
        nc.gpsimd.collective_compute(
            "AllGather", mybir.AluOpType.bypass,
            replica_groups=REPLICA_GROUPS,
            ins=[kin.opt()], outs=[kout.opt()],
        )
        vin = dram.tile([8, P, FREE], BF16, name="vin")
        for st in range(SL // P):
            nc.gpsimd.dma_start(vin[2 * st], vshard[st][:, 0:FREE])
            nc.gpsimd.dma_start(vin[2 * st + 1], vshard[st][:, FREE:2 * FREE])
        vout = dram.tile([R, 8, P, FREE], BF16, name="vout")
        nc.gpsimd.collective_compute(
            "AllGather", mybir.AluOpType.bypass,
            replica_groups=REPLICA_GROUPS,
            ins=[vin.opt()], outs=[vout.opt()],
        )

    # ---- Q projection (reuses freed weight slots) ----
    for eb in (0, 1):
        for d in range(DCH):
            t = w_pool.tile([P, FREE], BF16, name=f"w{eb}_{d}")
            nc.gpsimd.dma_start(t[:], wqkvT.ap()[P * d:P * (d + 1),
                                                 FREE * eb:FREE * (eb + 1)])
            wblk[(eb, d)] = t
    qt_pool = ctx.enter_context(tc.tile_pool(name="qt", bufs=1))
    qt = []
    with tc.tile_pool(name="projq_ps", bufs=4, space="PSUM") as projq_ps:
        for t in range(ET_Q):
            ps = projq_ps.tile([P, FREE], F32, name=f"psq{t}", tag="proj")
            eb = t // 4
            co = P * (t % 4)
            for d in range(DCH):
                nc.tensor.matmul(ps[:], wblk[(eb, d)][:, co:co + P], xt[d][:],
                                 start=(d == 0), stop=False)
            nc.tensor.matmul(ps[:], bqkv_sb[:, P * t:P * (t + 1)], ones_bf[:],
                             start=False, stop=True)
            q = qt_pool.tile([P, FREE], BF16, name=f"qt{t}")
            nc.scalar.copy(q[:], ps[:])
            qt.append(q)

    # ---- KV readback into attention layout (one DMA per tile) ----
    kv_pool = ctx.enter_context(tc.tile_pool(name="kv", bufs=1))
    kt = []
    for t in range(ET_Q):
        k = kv_pool.tile([P, S], BF16, name=f"kt{t}")
        nc.sync.dma_start(k.rearrange("p (r c) -> p r c", r=R),
                          kout[:, t].rearrange("r p c -> p r c"))
        kt.append(k)
    vt = []
    for g in range(NKK):
        r, i = g // 4, g % 4
        v = kv_pool.tile([P, H, HD + 1], BF16, name=f"vt{g}")
        for b2 in range(2):
            nc.sync.dma_start(
                v[:, 8 * b2:8 * (b2 + 1), 0:HD],
                vout[r, 2 * i + b2].rearrange("p (h d) -> p h d", d=HD))
        nc.vector.memset(v[:, :, HD:HD + 1], 1.0)
        vt.append(v)

    # ---- prefetch output-projection weights ----
    wo_pool = ctx.enter_context(tc.tile_pool(name="wo", bufs=1))
    wo = []
    for p_ in range(DCH):
        t = wo_pool.tile([P, D], BF16, name=f"wo{p_}")
        nc.sync.dma_start(t[:], woutT.ap()[P * p_:P * (p_ + 1), :])
        wo.append(t)

    # ---- attention ----
    attn_sb_pool = ctx.enter_context(tc.tile_pool(name="attnsb", bufs=1))
    small_pool = ctx.enter_context(tc.tile_pool(name="small", bufs=4))
    attn_sb = [attn_sb_pool.tile([P, FREE], BF16, name=f"attnsb{p_}")
               for p_ in range(H // 2)]
    GRP = 2  # kk-chunks per score-psum tile
    with tc.tile_pool(name="sc_ps", bufs=2, space="PSUM") as sc_ps, \
         tc.tile_pool(name="at_ps", bufs=2, space="PSUM") as at_ps, \
         tc.tile_pool(name="bc_ps", bufs=2, space="PSUM") as bc_ps, \
         tc.tile_pool(name="e_sb", bufs=4) as e_pool:
        pending = []
        pv_pending = []

        def normalize(h, at):
            # attn = attnT[0:64] * (1/Z) ; Z = attnT row 64
            koff = HD * (h % 2)
            rz = small_pool.tile([1, FREE], BF16, name=f"rz{h}", tag="rz")
            with nc.allow_low_precision(reason="softmax 1/Z scale in bf16"):
                nc.vector.reciprocal(rz[:], at[HD:HD + 1, :])
            bc = bc_ps.tile([HD, FREE], F32, name=f"bc{h}", tag="bc")
            nc.tensor.matmul(bc[:], ones_bf[:, :HD], rz[:],
                             start=True, stop=True)
            rzb = small_pool.tile([HD, FREE], F32, name=f"rzb{h}", tag="rzb")
            nc.vector.tensor_copy(rzb[:], bc[:])
            nc.vector.tensor_mul(attn_sb[h // 2][koff:koff + HD, :],
                                 at[0:HD, :], rzb[:])

        def attn_v(h, at, g, e):
            for j in range(GRP):
                kk = GRP * g + j
                nc.tensor.matmul(at[:], vt[kk][:, h, :],
                                 e[:, FREE * j:FREE * (j + 1)],
                                 start=(kk == 0), stop=(kk == NKK - 1))

        for h in range(H):
            ktile, koff = h // 2, HD * (h % 2)
            q_rhs = qt[ktile][koff:koff + HD, :]
            at = at_ps.tile([HD + 1, FREE], F32, name=f"at{h}", tag="at")
            for g in range(NKK // GRP):
                sc = sc_ps.tile([P, GRP * FREE], F32, name=f"sc{h}_{g}",
                                tag="sc")
                for j in range(GRP):
                    kk = GRP * g + j
                    nc.tensor.matmul(
                        sc[:, FREE * j:FREE * (j + 1)],
                        kt[ktile][koff:koff + HD, P * kk:P * (kk + 1)],
                        q_rhs, start=True, stop=True)
                e = e_pool.tile([P, GRP * FREE], BF16, name=f"e{h}_{g}",
                                tag="e")
                nc.scalar.activation(e[:], sc[:],
                                     mybir.ActivationFunctionType.Exp)
                # run the PV matmuls one group behind the scores stream, so
                # the PE never waits on the exp it just produced
                if pv_pending:
                    attn_v(*pv_pending.pop())
                pv_pending.append((h, at, g, e))
                if g == 2 and pending:
                    normalize(*pending.pop())
            pending.append((h, at))
        attn_v(*pv_pending.pop())
        normalize(*pending.pop())

    # ---- output projection ----
    with tc.tile_pool(name="out_ps", bufs=1, space="PSUM") as out_ps, \
         tc.tile_pool(name="out_sb", bufs=2) as out_sb_pool:
        ops = [[out_ps.tile([P, FREE], F32, name=f"op{st}_{eb}")
                for eb in range(2)] for st in range(SL // P)]
        for p_ in range(DCH):
            for st in range(SL // P):
                for eb in range(2):
                    nc.tensor.matmul(ops[st][eb],
                                     attn_sb[p_][:, P * st:P * (st + 1)],
                                     wo[p_][:, FREE * eb:FREE * (eb + 1)],
                                     start=(p_ == 0), stop=False)
        for st in range(SL // P):
            for eb in range(2):
                nc.tensor.matmul(ops[st][eb], ones_bf[:, :P],
                                 bout_sb[:, FREE * eb:FREE * (eb + 1)],
                                 start=False, stop=True)
            osb = out_sb_pool.tile([P, D], F32, name=f"osb{st}", tag="osb")
            for eb in range(2):
                nc.vector.tensor_copy(osb[:, FREE * eb:FREE * (eb + 1)],
                                      ops[st][eb])
            nc.sync.dma_start(out.ap()[P * st:P * (st + 1), :], osb[:])

    ctx.close()


_CACHE = {}


def _get_program():
    if "nc" not in _CACHE:
        _CACHE["nc"] = build_program()
    return _CACHE["nc"]


def prep_inputs(input_tensor, qkv_weight, qkv_bias, out_weight, out_bias):
    """Host-side shard + transpose + cast. Returns in_maps for 8 cores."""
    x = np.asarray(input_tensor, np.float32)
    wqkv = np.asarray(qkv_weight, np.float32).copy()
    bq = np.asarray(qkv_bias, np.float32).copy()
    scale = 1.0 / np.sqrt(np.float32(HD))
    wqkv[:D] *= scale
    bq[:D] *= scale
    bf = ml_dtypes.bfloat16
    wqkvT = np.ascontiguousarray(wqkv.T).astype(bf)
    bqkv = bq.reshape(1, 3 * D).astype(bf)
    woutT = np.ascontiguousarray(np.asarray(out_weight, np.float32).T).astype(bf)
    bout = np.asarray(out_bias, np.float32).reshape(1, D).astype(bf)
    in_maps = []
    for c in range(N_CORES):
        b, j = c // R, c % R
        xTc = np.ascontiguousarray(x[b, SL * j:SL * (j + 1), :].T).astype(bf)
        in_maps.append({"xT": xTc, "wqkvT": wqkvT, "bqkv": bqkv,
                        "woutT": woutT, "bout": bout})
    return in_maps


def kernel(input_tensor, qkv_weight, qkv_bias, out_weight, out_bias,
           **run_kwargs):
    nc = _get_program()
    in_maps = prep_inputs(input_tensor, qkv_weight, qkv_bias, out_weight,
                          out_bias)
    res = run_bass_kernel_spmd(nc, in_maps, core_ids=list(range(N_CORES)),
                               **run_kwargs)
    full = np.empty((B, S, D), np.float32)
    for c in range(N_CORES):
        b, j = c // R, c % R
        full[b, SL * j:SL * (j + 1), :] = res.results[c]["out"]
    if run_kwargs:
        kernel.last_results = res
    return full


# revision 16
# speedup vs baseline: 1.2258x; 1.1087x over previous
"""Multi-head attention (B=2, S=2048, D=1024, H=16) on 8 Trainium2 NeuronCores.

Sharding: core c handles (batch b=c//4, query chunk j=c%4 of 512 rows).
 - Each core computes K^T / V for its WHOLE batch locally (weights replicated,
   pre-transposed + bf16-cast on host; softmax scale folded into W_q) — no
   collectives, the PE stays continuously busy so the HAM clock-gate stays
   warm.
 - Q projected for the core's own 512 rows only.
 - Attention (all 16 heads, 512 queries x 2048 keys):
   scoresT = K_h @ Q_h^T  ->  exp on ACT  ->  attnT = [V_h|1]^T @ E
   (ones column gives the softmax denominator Z in row 64 of attnT psum).
 - Q/K biases folded into the projection casts (ACT Identity per-partition
   bias); V bias folded into the output bias on host (sum(probs) == 1).
 - Output projection local per 512-row chunk; final output assembled on host.
"""

import numpy as np
import ml_dtypes

import concourse.bass as bass
import concourse.mybir as mybir
import concourse.tile as tile
from concourse import bacc
from concourse.bass_utils import run_bass_kernel_spmd

BF16 = mybir.dt.bfloat16
F32 = mybir.dt.float32
AF = mybir.ActivationFunctionType

B, S, D = 2, 2048, 1024
H, HD = 16, 64
N_CORES = 8
R = 4            # cores per batch
SL = S // R      # local query rows per core (512)
P = 128
DCH = D // P     # 8 d-chunks
NKK = S // P     # 16 key chunks
ET = D // P      # 8 feature tiles per projection
FREE = 512


def build_program():
    nc = bacc.Bacc("TRN2", target_bir_lowering=False, debug=False,
                   num_devices=N_CORES)

    xT = nc.dram_tensor("xT", [D, S], BF16, kind="ExternalInput")
    xqT = nc.dram_tensor("xqT", [D, SL], BF16, kind="ExternalInput")
    wqkvT = nc.dram_tensor("wqkvT", [D, 3 * D], BF16, kind="ExternalInput")
    bqk = nc.dram_tensor("bqk", [P, 16], BF16, kind="ExternalInput")
    woutT = nc.dram_tensor("woutT", [D, D], BF16, kind="ExternalInput")
    bout = nc.dram_tensor("bout", [1, D], BF16, kind="ExternalInput")
    out = nc.dram_tensor("out", [SL, D], F32, kind="ExternalOutput")

    with tile.TileContext(nc) as tc:
        _build(nc, tc, xT, xqT, wqkvT, bqk, woutT, bout, out)
    nc.compile()
    return nc


def _build(nc, tc, xT, xqT, wqkvT, bqk, woutT, bout, out):
    from contextlib import ExitStack

    ctx = ExitStack()
    consts = ctx.enter_context(tc.tile_pool(name="consts", bufs=1))

    # ---- constants ----
    ones_bf = consts.tile([1, FREE], BF16, name="ones_bf")
    nc.vector.memset(ones_bf[:], 1.0)
    bqk_sb = consts.tile([P, 16], BF16, name="bqk_sb")
    nc.sync.dma_start(bqk_sb[:], bqk.ap())
    bout_sb = consts.tile([1, D], BF16, name="bout_sb")
    nc.sync.dma_start(bout_sb[:], bout.ap())

    # ---- resident input tiles ----
    xt_pool = ctx.enter_context(tc.tile_pool(name="xt", bufs=1))
    xt = []
    for i in range(DCH):
        t = xt_pool.tile([P, S], BF16, name=f"xt{i}")
        nc.sync.dma_start(t[:], xT.ap()[P * i:P * (i + 1), :])
        xt.append(t)
    xq = []
    for i in range(DCH):
        t = xt_pool.tile([P, SL], BF16, name=f"xq{i}")
        nc.sync.dma_start(t[:], xqT.ap()[P * i:P * (i + 1), :])
        xq.append(t)

    # ---- weight stream (K, V blocks first, then Q) ----
    w_pool = ctx.enter_context(tc.tile_pool(name="wq", bufs=16))
    wblk = {}

    def load_w(ebs):
        for eb in ebs:
            for d in range(DCH):
                t = w_pool.tile([P, FREE], BF16, name=f"w{eb}_{d}", tag="w")
                nc.gpsimd.dma_start(t[:], wqkvT.ap()[P * d:P * (d + 1),
                                                     FREE * eb:FREE * (eb + 1)])
                wblk[(eb, d)] = t

    kv_pool = ctx.enter_context(tc.tile_pool(name="kv", bufs=1))
    kt = [kv_pool.tile([P, S], BF16, name=f"kt{t}") for t in range(ET)]
    vt = [kv_pool.tile([P, H, HD + 1], BF16, name=f"vt{g}")
          for g in range(NKK)]
    for g in range(NKK):
        nc.vector.memset(vt[g][:, :, HD:HD + 1], 1.0)
    qt_pool = ctx.enter_context(tc.tile_pool(name="qt", bufs=1))
    qt = [qt_pool.tile([P, FREE], BF16, name=f"qt{t}") for t in range(ET)]

    # ---- K^T projection, full batch: out[e, s] ----
    load_w((2, 3))
    with tc.tile_pool(name="projk_ps", bufs=4, space="PSUM") as ps_pool:
        for t in range(ET):
            eb = 2 + t // 4
            co = P * (t % 4)
            for sch in range(4):
                ps = ps_pool.tile([P, FREE], F32, name=f"psk{t}_{sch}",
                                  tag="proj")
                for d in range(DCH):
                    nc.tensor.matmul(ps[:],
                                     wblk[(eb, d)][:, co:co + P],
                                     xt[d][:, FREE * sch:FREE * (sch + 1)],
                                     start=(d == 0), stop=(d == DCH - 1))
                # cast + K-bias (per-partition) fused on ACT
                nc.scalar.activation(kt[t][:, FREE * sch:FREE * (sch + 1)],
                                     ps[:], AF.Identity,
                                     bias=bqk_sb[:, 8 + t:9 + t])

    # ---- V projection, full batch, natural: out[s, e] ----
    load_w((4, 5))
    with tc.tile_pool(name="projv_ps", bufs=8, space="PSUM") as ps_pool:
        for st in range(NKK):
            for eb in range(2):
                ps = ps_pool.tile([P, FREE], F32, name=f"psv{st}_{eb}",
                                  tag="proj")
                for d in range(DCH):
                    nc.tensor.matmul(ps[:],
                                     xt[d][:, P * st:P * (st + 1)],
                                     wblk[(4 + eb, d)][:],
                                     start=(d == 0), stop=(d == DCH - 1))
                # cast to the [V|1] attention layout on DVE (no bias:
                # V-bias is folded into the output bias on host)
                nc.vector.tensor_copy(
                    vt[st][:, 8 * eb:8 * (eb + 1), 0:HD],
                    ps.rearrange("p (h d) -> p h d", d=HD))

    # ---- Q projection (own 512 rows): out[e, q] ----
    load_w((0, 1))
    with tc.tile_pool(name="projq_ps", bufs=4, space="PSUM") as ps_pool:
        for t in range(ET):
            eb = t // 4
            co = P * (t % 4)
            ps = ps_pool.tile([P, FREE], F32, name=f"psq{t}", tag="proj")
            for d in range(DCH):
                nc.tensor.matmul(ps[:], wblk[(eb, d)][:, co:co + P], xq[d][:],
                                 start=(d == 0), stop=(d == DCH - 1))
            nc.scalar.activation(qt[t][:], ps[:], AF.Identity,
                                 bias=bqk_sb[:, t:t + 1])

    # ---- prefetch output-projection weights ----
    wo_pool = ctx.enter_context(tc.tile_pool(name="wo", bufs=1))
    wo = []
    for p_ in range(DCH):
        t = wo_pool.tile([P, D], BF16, name=f"wo{p_}")
        nc.sync.dma_start(t[:], woutT.ap()[P * p_:P * (p_ + 1), :])
        wo.append(t)

    # ---- attention ----
    attn_sb_pool = ctx.enter_context(tc.tile_pool(name="attnsb", bufs=1))
    small_pool = ctx.enter_context(tc.tile_pool(name="small", bufs=4))
    attn_sb = [attn_sb_pool.tile([P, FREE], BF16, name=f"attnsb{p_}")
               for p_ in range(H // 2)]
    GRP = 2  # kk-chunks per score-psum tile
    with tc.tile_pool(name="sc_ps", bufs=2, space="PSUM") as sc_ps, \
         tc.tile_pool(name="at_ps", bufs=2, space="PSUM") as at_ps, \
         tc.tile_pool(name="bc_ps", bufs=2, space="PSUM") as bc_ps, \
         tc.tile_pool(name="e_sb", bufs=4) as e_pool:
        pending = []
        pv_pending = []

        def normalize(h, at):
            # attn = attnT[0:64] * (1/Z) ; Z = attnT row 64
            koff = HD * (h % 2)
            rz = small_pool.tile([1, FREE], BF16, name=f"rz{h}", tag="rz")
            with nc.allow_low_precision(reason="softmax 1/Z scale in bf16"):
                nc.vector.reciprocal(rz[:], at[HD:HD + 1, :])
            bc = bc_ps.tile([HD, FREE], F32, name=f"bc{h}", tag="bc")
            nc.tensor.matmul(bc[:], ones_bf[:, :HD], rz[:],
                             start=True, stop=True)
            rzb = small_pool.tile([HD, FREE], F32, name=f"rzb{h}", tag="rzb")
            nc.vector.tensor_copy(rzb[:], bc[:])
            nc.vector.tensor_mul(attn_sb[h // 2][koff:koff + HD, :],
                                 at[0:HD, :], rzb[:])

        def attn_v(h, at, g, e):
            for j in range(GRP):
                kk = GRP * g + j
                nc.tensor.matmul(at[:], vt[kk][:, h, :],
                                 e[:, FREE * j:FREE * (j + 1)],
                                 start=(kk == 0), stop=(kk == NKK - 1))

        for h in range(H):
            ktile, koff = h // 2, HD * (h % 2)
            q_rhs = qt[ktile][koff:koff + HD, :]
            at = at_ps.tile([HD + 1, FREE], F32, name=f"at{h}", tag="at")
            for g in range(NKK // GRP):
                sc = sc_ps.tile([P, GRP * FREE], F32, name=f"sc{h}_{g}",
                                tag="sc")
                for j in range(GRP):
                    kk = GRP * g + j
                    nc.tensor.matmul(
                        sc[:, FREE * j:FREE * (j + 1)],
                        kt[ktile][koff:koff + HD, P * kk:P * (kk + 1)],
                        q_rhs, start=True, stop=True)
                e = e_pool.tile([P, GRP * FREE], BF16, name=f"e{h}_{g}",
                                tag="e")
                nc.scalar.activation(e[:], sc[:], AF.Exp)
                # run the PV matmuls one group behind the scores stream, so
                # the PE never waits on the exp it just produced
                if pv_pending:
                    attn_v(*pv_pending.pop())
                pv_pending.append((h, at, g, e))
                if g == 2 and pending:
                    normalize(*pending.pop())
            pending.append((h, at))
        attn_v(*pv_pending.pop())
        normalize(*pending.pop())

    # ---- output projection ----
    with tc.tile_pool(name="out_ps", bufs=1, space="PSUM") as out_ps, \
         tc.tile_pool(name="out_sb", bufs=2) as out_sb_pool:
        ops = [[out_ps.tile([P, FREE], F32, name=f"op{st}_{eb}")
                for eb in range(2)] for st in range(SL // P)]
        for p_ in range(DCH):
            for st in range(SL // P):
                for eb in range(2):
                    nc.tensor.matmul(ops[st][eb],
                                     attn_sb[p_][:, P * st:P * (st + 1)],
                                     wo[p_][:, FREE * eb:FREE * (eb + 1)],
                                     start=(p_ == 0), stop=False)
        for st in range(SL // P):
            for eb in range(2):
                nc.tensor.matmul(ops[st][eb], ones_bf[:, :P],
                                 bout_sb[:, FREE * eb:FREE * (eb + 1)],
                                 start=False, stop=True)
            osb = out_sb_pool.tile([P, D], F32, name=f"osb{st}", tag="osb")
            for eb in range(2):
                nc.vector.tensor_copy(osb[:, FREE * eb:FREE * (eb + 1)],
                                      ops[st][eb])
            nc.sync.dma_start(out.ap()[P * st:P * (st + 1), :], osb[:])

    ctx.close()


_CACHE = {}


def _get_program():
    if "nc" not in _CACHE:
        _CACHE["nc"] = build_program()
    return _CACHE["nc"]


def prep_inputs(input_tensor, qkv_weight, qkv_bias, out_weight, out_bias):
    """Host-side shard + transpose + cast. Returns in_maps for 8 cores."""
    x = np.asarray(input_tensor, np.float32)
    wqkv = np.asarray(qkv_weight, np.float32).copy()
    bq = np.asarray(qkv_bias, np.float32).copy()
    wout = np.asarray(out_weight, np.float32)
    scale = 1.0 / np.sqrt(np.float32(HD))
    wqkv[:D] *= scale
    bq[:D] *= scale
    bf = ml_dtypes.bfloat16
    wqkvT = np.ascontiguousarray(wqkv.T).astype(bf)
    # Q/K biases, column-major per 128-feature tile: bqk[:, t] = bias tile t
    bqk = np.ascontiguousarray(bq[:2 * D].reshape(16, P).T).astype(bf)
    woutT = np.ascontiguousarray(wout.T).astype(bf)
    # V bias folded into output bias: probs @ (V + b_v) = probs @ V + b_v
    bout_eff = np.asarray(out_bias, np.float32) + wout @ bq[2 * D:]
    bout = bout_eff.reshape(1, D).astype(bf)
    xTb = [np.ascontiguousarray(x[b].T).astype(bf) for b in range(B)]
    in_maps = []
    for c in range(N_CORES):
        b, j = c // R, c % R
        xqT = np.ascontiguousarray(xTb[b][:, SL * j:SL * (j + 1)])
        in_maps.append({"xT": xTb[b], "xqT": xqT, "wqkvT": wqkvT,
                        "bqk": bqk, "woutT": woutT, "bout": bout})
    return in_maps


def kernel(input_tensor, qkv_weight, qkv_bias, out_weight, out_bias,
           **run_kwargs):
    nc = _get_program()
    in_maps = prep_inputs(input_tensor, qkv_weight, qkv_bias, out_weight,
                          out_bias)
    res = run_bass_kernel_spmd(nc, in_maps, core_ids=list(range(N_CORES)),
                               **run_kwargs)
    full = np.empty((B, S, D), np.float32)
    for c in range(N_CORES):
        b, j = c // R, c % R
        full[b, SL * j:SL * (j + 1), :] = res.results[c]["out"]
    if run_kwargs:
        kernel.last_results = res
    return full


# revision 20
# speedup vs baseline: 1.4657x; 1.1957x over previous
"""Multi-head attention (B=2, S=2048, D=1024, H=16) on 8 Trainium2 NeuronCores.

Sharding: core c handles (batch b=c//4, query chunk j=c%4 of 512 rows).
 - Each core computes K^T / V for its WHOLE batch locally (weights replicated,
   pre-transposed + bf16-cast on host; softmax scale folded into W_q) — no
   collectives, the PE stays continuously busy so the HAM clock-gate stays
   warm.
 - Q projected for the core's own 512 rows only.
 - Attention (all 16 heads, 512 queries x 2048 keys):
   scoresT = K_h @ Q_h^T  ->  exp on ACT  ->  attnT = [V_h|1]^T @ E
   (ones column gives the softmax denominator Z in row 64 of attnT psum).
 - Q/K biases folded into the projection casts (ACT Identity per-partition
   bias); V bias folded into the output bias on host (sum(probs) == 1).
 - Output projection local per 512-row chunk; final output assembled on host.
"""

import numpy as np
import ml_dtypes

import concourse.bass as bass
import concourse.mybir as mybir
import concourse.tile as tile
from concourse import bacc
from concourse.bass_utils import run_bass_kernel_spmd

BF16 = mybir.dt.bfloat16
F32 = mybir.dt.float32
AF = mybir.ActivationFunctionType

B, S, D = 2, 2048, 1024
H, HD = 16, 64
N_CORES = 8
R = 4            # cores per batch
SL = S // R      # local query rows per core (512)
P = 128
DCH = D // P     # 8 d-chunks
NKK = S // P     # 16 key chunks
ET = D // P      # 8 feature tiles per projection
FREE = 512


def build_program():
    nc = bacc.Bacc("TRN2", target_bir_lowering=False, debug=False,
                   num_devices=N_CORES)

    xT = nc.dram_tensor("xT", [D, S], BF16, kind="ExternalInput")
    xqT = nc.dram_tensor("xqT", [D, SL], BF16, kind="ExternalInput")
    wqkvT = nc.dram_tensor("wqkvT", [D, 3 * D], BF16, kind="ExternalInput")
    bqk = nc.dram_tensor("bqk", [P, 16], BF16, kind="ExternalInput")
    woutT = nc.dram_tensor("woutT", [D, D], BF16, kind="ExternalInput")
    bout = nc.dram_tensor("bout", [1, D], BF16, kind="ExternalInput")
    out = nc.dram_tensor("out", [SL, D], F32, kind="ExternalOutput")

    with tile.TileContext(nc) as tc:
        _build(nc, tc, xT, xqT, wqkvT, bqk, woutT, bout, out)
    nc.compile()
    return nc


def _build(nc, tc, xT, xqT, wqkvT, bqk, woutT, bout, out):
    from contextlib import ExitStack

    ctx = ExitStack()
    consts = ctx.enter_context(tc.tile_pool(name="consts", bufs=1))

    # ---- constants ----
    ones_bf = consts.tile([1, FREE], BF16, name="ones_bf")
    nc.vector.memset(ones_bf[:], 1.0)
    bqk_sb = consts.tile([P, 16], BF16, name="bqk_sb")
    nc.sync.dma_start(bqk_sb[:], bqk.ap())
    bout_sb = consts.tile([1, D], BF16, name="bout_sb")
    nc.sync.dma_start(bout_sb[:], bout.ap())

    # ---- resident input tiles ----
    xt_pool = ctx.enter_context(tc.tile_pool(name="xt", bufs=1))
    xt = []
    for i in range(DCH):
        t = xt_pool.tile([P, S], BF16, name=f"xt{i}")
        nc.sync.dma_start(t[:], xT.ap()[P * i:P * (i + 1), :])
        xt.append(t)
    xq = []
    for i in range(DCH):
        t = xt_pool.tile([P, SL], BF16, name=f"xq{i}")
        nc.sync.dma_start(t[:], xqT.ap()[P * i:P * (i + 1), :])
        xq.append(t)

    # ---- weight stream (K, V blocks first, then Q) ----
    w_pool = ctx.enter_context(tc.tile_pool(name="wq", bufs=16))
    wblk = {}

    def load_w(ebs):
        for eb in ebs:
            for d in range(DCH):
                t = w_pool.tile([P, FREE], BF16, name=f"w{eb}_{d}", tag="w")
                nc.gpsimd.dma_start(t[:], wqkvT.ap()[P * d:P * (d + 1),
                                                     FREE * eb:FREE * (eb + 1)])
                wblk[(eb, d)] = t

    # V layout: per key-tile, 16 heads x (64 V-features + ones col) packed at
    # stride 65, plus 64 zero columns of tail pad so a 128-wide stationary
    # slice [65h : 65h+128] is always in bounds. Columns 65..127 of that
    # slice hit neighbor-head data; they only feed psum rows 65..127 which
    # are never read. Full 128x128 stationary keeps the PE clock-gate warm.
    VW = H * (HD + 1) + HD  # 1104
    kv_pool = ctx.enter_context(tc.tile_pool(name="kv", bufs=1))
    kt = [kv_pool.tile([P, S], BF16, name=f"kt{t}") for t in range(ET)]
    vt = [kv_pool.tile([P, VW], BF16, name=f"vt{g}") for g in range(NKK)]
    for g in range(NKK):
        v3 = vt[g][:, 0:H * (HD + 1)].rearrange("p (h c) -> p h c", c=HD + 1)
        nc.vector.memset(v3[:, :, HD:HD + 1], 1.0)
        nc.vector.memset(vt[g][:, H * (HD + 1):VW], 0.0)
    # Q, zero-padded per head: head h occupies partitions 64*(h%2)..+64 of
    # qz[h], the other 64 partitions are zero -> scores matmul can use the
    # full 128-partition K^T pair tile as stationary (K=128, stays warm).
    qt_pool = ctx.enter_context(tc.tile_pool(name="qt", bufs=1))
    qz = [qt_pool.tile([P, FREE], BF16, name=f"qz{h}") for h in range(H)]
    for h in range(H):
        off = HD * ((h + 1) % 2)
        nc.vector.memset(qz[h][off:off + HD, :], 0.0)

    # ---- K^T projection, full batch: out[e, s] ----
    load_w((2, 3))
    with tc.tile_pool(name="projk_ps", bufs=4, space="PSUM") as ps_pool:
        for t in range(ET):
            eb = 2 + t // 4
            co = P * (t % 4)
            for sch in range(4):
                ps = ps_pool.tile([P, FREE], F32, name=f"psk{t}_{sch}",
                                  tag="proj")
                for d in range(DCH):
                    nc.tensor.matmul(ps[:],
                                     wblk[(eb, d)][:, co:co + P],
                                     xt[d][:, FREE * sch:FREE * (sch + 1)],
                                     start=(d == 0), stop=(d == DCH - 1))
                # cast + K-bias (per-partition) fused on ACT
                nc.scalar.activation(kt[t][:, FREE * sch:FREE * (sch + 1)],
                                     ps[:], AF.Identity,
                                     bias=bqk_sb[:, 8 + t:9 + t])

    # ---- V projection, full batch, natural: out[s, e] ----
    load_w((4, 5))
    with tc.tile_pool(name="projv_ps", bufs=8, space="PSUM") as ps_pool:
        for st in range(NKK):
            for eb in range(2):
                ps = ps_pool.tile([P, FREE], F32, name=f"psv{st}_{eb}",
                                  tag="proj")
                for d in range(DCH):
                    nc.tensor.matmul(ps[:],
                                     xt[d][:, P * st:P * (st + 1)],
                                     wblk[(4 + eb, d)][:],
                                     start=(d == 0), stop=(d == DCH - 1))
                # cast to the [V|1] attention layout on DVE (no bias:
                # V-bias is folded into the output bias on host)
                v3 = vt[st][:, 0:H * (HD + 1)].rearrange(
                    "p (h c) -> p h c", c=HD + 1)
                nc.vector.tensor_copy(
                    v3[:, 8 * eb:8 * (eb + 1), 0:HD],
                    ps.rearrange("p (h d) -> p h d", d=HD))

    # ---- Q projection (own 512 rows): out[e, q] ----
    load_w((0, 1))
    with tc.tile_pool(name="projq_ps", bufs=4, space="PSUM") as ps_pool:
        for t in range(ET):
            eb = t // 4
            co = P * (t % 4)
            ps = ps_pool.tile([P, FREE], F32, name=f"psq{t}", tag="proj")
            for d in range(DCH):
                nc.tensor.matmul(ps[:], wblk[(eb, d)][:, co:co + P], xq[d][:],
                                 start=(d == 0), stop=(d == DCH - 1))
            nc.scalar.activation(qz[2 * t][0:HD, :], ps[0:HD, :], AF.Identity,
                                 bias=bqk_sb[0:HD, t:t + 1])
            nc.scalar.activation(qz[2 * t + 1][HD:P, :], ps[HD:P, :],
                                 AF.Identity, bias=bqk_sb[HD:P, t:t + 1])

    # ---- prefetch output-projection weights ----
    wo_pool = ctx.enter_context(tc.tile_pool(name="wo", bufs=1))
    wo = []
    for p_ in range(DCH):
        t = wo_pool.tile([P, D], BF16, name=f"wo{p_}")
        nc.sync.dma_start(t[:], woutT.ap()[P * p_:P * (p_ + 1), :])
        wo.append(t)

    # ---- attention ----
    attn_sb_pool = ctx.enter_context(tc.tile_pool(name="attnsb", bufs=1))
    small_pool = ctx.enter_context(tc.tile_pool(name="small", bufs=4))
    attn_sb = [attn_sb_pool.tile([P, FREE], BF16, name=f"attnsb{p_}")
               for p_ in range(H // 2)]
    GRP = 2  # kk-chunks per score-psum tile
    with tc.tile_pool(name="sc_ps", bufs=2, space="PSUM") as sc_ps, \
         tc.tile_pool(name="at_ps", bufs=2, space="PSUM") as at_ps, \
         tc.tile_pool(name="bc_ps", bufs=2, space="PSUM") as bc_ps, \
         tc.tile_pool(name="e_sb", bufs=4) as e_pool:
        pending = []
        pv_pending = []

        def normalize(h, at):
            # attn = attnT[0:64] * (1/Z) ; Z = attnT row 64
            koff = HD * (h % 2)
            rz = small_pool.tile([1, FREE], BF16, name=f"rz{h}", tag="rz")
            with nc.allow_low_precision(reason="softmax 1/Z scale in bf16"):
                nc.vector.reciprocal(rz[:], at[HD:HD + 1, :])
            bc = bc_ps.tile([HD, FREE], F32, name=f"bc{h}", tag="bc")
            nc.tensor.matmul(bc[:], ones_bf[:, :HD], rz[:],
                             start=True, stop=True)
            rzb = small_pool.tile([HD, FREE], F32, name=f"rzb{h}", tag="rzb")
            nc.vector.tensor_copy(rzb[:], bc[:])
            nc.vector.tensor_mul(attn_sb[h // 2][koff:koff + HD, :],
                                 at[0:HD, :], rzb[:])

        def attn_v(h, at, g, e):
            for j in range(GRP):
                kk = GRP * g + j
                nc.tensor.matmul(at[:], vt[kk][:, 65 * h:65 * h + P],
                                 e[:, FREE * j:FREE * (j + 1)],
                                 start=(kk == 0), stop=(kk == NKK - 1))

        for h in range(H):
            ktile = h // 2
            q_rhs = qz[h][:]
            at = at_ps.tile([P, FREE], F32, name=f"at{h}", tag="at")
            for g in range(NKK // GRP):
                sc = sc_ps.tile([P, GRP * FREE], F32, name=f"sc{h}_{g}",
                                tag="sc")
                for j in range(GRP):
                    kk = GRP * g + j
                    nc.tensor.matmul(
                        sc[:, FREE * j:FREE * (j + 1)],
                        kt[ktile][:, P * kk:P * (kk + 1)],
                        q_rhs, start=True, stop=True)
                e = e_pool.tile([P, GRP * FREE], BF16, name=f"e{h}_{g}",
                                tag="e")
                nc.scalar.activation(e[:], sc[:], AF.Exp)
                # run the PV matmuls one group behind the scores stream, so
                # the PE never waits on the exp it just produced
                if pv_pending:
                    attn_v(*pv_pending.pop())
                pv_pending.append((h, at, g, e))
                if g == 2 and pending:
                    normalize(*pending.pop())
            pending.append((h, at))
        attn_v(*pv_pending.pop())
        normalize(*pending.pop())

    # ---- output projection ----
    with tc.tile_pool(name="out_ps", bufs=1, space="PSUM") as out_ps, \
         tc.tile_pool(name="out_sb", bufs=2) as out_sb_pool:
        ops = [[out_ps.tile([P, FREE], F32, name=f"op{st}_{eb}")
                for eb in range(2)] for st in range(SL // P)]
        for p_ in range(DCH):
            for st in range(SL // P):
                for eb in range(2):
                    nc.tensor.matmul(ops[st][eb],
                                     attn_sb[p_][:, P * st:P * (st + 1)],
                                     wo[p_][:, FREE * eb:FREE * (eb + 1)],
                                     start=(p_ == 0), stop=False)
        for st in range(SL // P):
            for eb in range(2):
                nc.tensor.matmul(ops[st][eb], ones_bf[:, :P],
                                 bout_sb[:, FREE * eb:FREE * (eb + 1)],
                                 start=False, stop=True)
            osb = out_sb_pool.tile([P, D], F32, name=f"osb{st}", tag="osb")
            for eb in range(2):
                nc.vector.tensor_copy(osb[:, FREE * eb:FREE * (eb + 1)],
                                      ops[st][eb])
            nc.sync.dma_start(out.ap()[P * st:P * (st + 1), :], osb[:])

    ctx.close()


_CACHE = {}


def _get_program():
    if "nc" not in _CACHE:
        _CACHE["nc"] = build_program()
    return _CACHE["nc"]


def prep_inputs(input_tensor, qkv_weight, qkv_bias, out_weight, out_bias):
    """Host-side shard + transpose + cast. Returns in_maps for 8 cores."""
    x = np.asarray(input_tensor, np.float32)
    wqkv = np.asarray(qkv_weight, np.float32).copy()
    bq = np.asarray(qkv_bias, np.float32).copy()
    wout = np.asarray(out_weight, np.float32)
    scale = 1.0 / np.sqrt(np.float32(HD))
    wqkv[:D] *= scale
    bq[:D] *= scale
    bf = ml_dtypes.bfloat16
    wqkvT = np.ascontiguousarray(wqkv.T).astype(bf)
    # Q/K biases, column-major per 128-feature tile: bqk[:, t] = bias tile t
    bqk = np.ascontiguousarray(bq[:2 * D].reshape(16, P).T).astype(bf)
    woutT = np.ascontiguousarray(wout.T).astype(bf)
    # V bias folded into output bias: probs @ (V + b_v) = probs @ V + b_v
    bout_eff = np.asarray(out_bias, np.float32) + wout @ bq[2 * D:]
    bout = bout_eff.reshape(1, D).astype(bf)
    xTb = [np.ascontiguousarray(x[b].T).astype(bf) for b in range(B)]
    in_maps = []
    for c in range(N_CORES):
        b, j = c // R, c % R
        xqT = np.ascontiguousarray(xTb[b][:, SL * j:SL * (j + 1)])
        in_maps.append({"xT": xTb[b], "xqT": xqT, "wqkvT": wqkvT,
                        "bqk": bqk, "woutT": woutT, "bout": bout})
    return in_maps


def kernel(input_tensor, qkv_weight, qkv_bias, out_weight, out_bias,
           **run_kwargs):
    nc = _get_program()
    in_maps = prep_inputs(input_tensor, qkv_weight, qkv_bias, out_weight,
                          out_bias)
    res = run_bass_kernel_spmd(nc, in_maps, core_ids=list(range(N_CORES)),
                               **run_kwargs)
    full = np.empty((B, S, D), np.float32)
    for c in range(N_CORES):
        b, j = c // R, c % R
        full[b, SL * j:SL * (j + 1), :] = res.results[c]["out"]
    if run_kwargs:
        kernel.last_results = res
    return full


# revision 23
# speedup vs baseline: 1.5155x; 1.0340x over previous
"""Multi-head attention (B=2, S=2048, D=1024, H=16) on 8 Trainium2 NeuronCores.

Sharding: core c handles (batch b=c//4, query chunk j=c%4 of 512 rows).
 - Each core computes K^T / V for its WHOLE batch locally (weights replicated,
   pre-transposed + bf16-cast on host; softmax scale folded into W_q) — no
   collectives, the PE stays continuously busy so the HAM clock-gate stays
   warm.
 - Q projected for the core's own 512 rows only.
 - Attention (all 16 heads, 512 queries x 2048 keys):
   scoresT = K_h @ Q_h^T  ->  exp on ACT  ->  attnT = [V_h|1]^T @ E
   (ones column gives the softmax denominator Z in row 64 of attnT psum).
 - Q/K biases folded into the projection casts (ACT Identity per-partition
   bias); V bias folded into the output bias on host (sum(probs) == 1).
 - Output projection local per 512-row chunk; final output assembled on host.
"""

import numpy as np
import ml_dtypes

import concourse.bass as bass
import concourse.mybir as mybir
import concourse.tile as tile
from concourse import bacc
from concourse.bass_utils import run_bass_kernel_spmd

BF16 = mybir.dt.bfloat16
F32 = mybir.dt.float32
AF = mybir.ActivationFunctionType

B, S, D = 2, 2048, 1024
H, HD = 16, 64
N_CORES = 8
R = 4            # cores per batch
SL = S // R      # local query rows per core (512)
P = 128
DCH = D // P     # 8 d-chunks
NKK = S // P     # 16 key chunks
ET = D // P      # 8 feature tiles per projection
FREE = 512


def build_program():
    nc = bacc.Bacc("TRN2", target_bir_lowering=False, debug=False,
                   num_devices=N_CORES)

    xT = nc.dram_tensor("xT", [D, S], BF16, kind="ExternalInput")
    xqT = nc.dram_tensor("xqT", [D, SL], BF16, kind="ExternalInput")
    wqkvT = nc.dram_tensor("wqkvT", [D, 3 * D], BF16, kind="ExternalInput")
    bqk = nc.dram_tensor("bqk", [P, 16], BF16, kind="ExternalInput")
    woutT = nc.dram_tensor("woutT", [D, D], BF16, kind="ExternalInput")
    bout = nc.dram_tensor("bout", [1, D], BF16, kind="ExternalInput")
    out = nc.dram_tensor("out", [SL, D], F32, kind="ExternalOutput")

    with tile.TileContext(nc) as tc:
        _build(nc, tc, xT, xqT, wqkvT, bqk, woutT, bout, out)
    nc.compile()
    return nc


def _build(nc, tc, xT, xqT, wqkvT, bqk, woutT, bout, out):
    from contextlib import ExitStack

    ctx = ExitStack()
    consts = ctx.enter_context(tc.tile_pool(name="consts", bufs=1))

    # ---- constants ----
    ones_bf = consts.tile([1, FREE], BF16, name="ones_bf")
    nc.vector.memset(ones_bf[:], 1.0)
    bqk_sb = consts.tile([P, 16], BF16, name="bqk_sb")
    nc.sync.dma_start(bqk_sb[:], bqk.ap())
    bout_sb = consts.tile([1, D], BF16, name="bout_sb")
    nc.sync.dma_start(bout_sb[:], bout.ap())

    # ---- resident input tiles ----
    xt_pool = ctx.enter_context(tc.tile_pool(name="xt", bufs=1))
    xt = []
    for i in range(DCH):
        t = xt_pool.tile([P, S], BF16, name=f"xt{i}")
        nc.sync.dma_start(t[:], xT.ap()[P * i:P * (i + 1), :])
        xt.append(t)
    xq = []
    for i in range(DCH):
        t = xt_pool.tile([P, SL], BF16, name=f"xq{i}")
        nc.sync.dma_start(t[:], xqT.ap()[P * i:P * (i + 1), :])
        xq.append(t)

    # ---- weight stream (K, V blocks first, then Q) ----
    w_pool = ctx.enter_context(tc.tile_pool(name="wq", bufs=16))
    wblk = {}

    def load_w(ebs):
        for eb in ebs:
            for d in range(DCH):
                t = w_pool.tile([P, FREE], BF16, name=f"w{eb}_{d}", tag="w")
                nc.gpsimd.dma_start(t[:], wqkvT.ap()[P * d:P * (d + 1),
                                                     FREE * eb:FREE * (eb + 1)])
                wblk[(eb, d)] = t

    # V layout: per key-tile, 16 heads x (64 V-features + ones col) packed at
    # stride 65, plus 64 zero columns of tail pad so a 128-wide stationary
    # slice [65h : 65h+128] is always in bounds. Columns 65..127 of that
    # slice hit neighbor-head data; they only feed psum rows 65..127 which
    # are never read. Full 128x128 stationary keeps the PE clock-gate warm.
    VW = H * (HD + 1) + HD  # 1104
    kv_pool = ctx.enter_context(tc.tile_pool(name="kv", bufs=1))
    kt = [kv_pool.tile([P, S], BF16, name=f"kt{t}") for t in range(ET)]
    vt = [kv_pool.tile([P, VW], BF16, name=f"vt{g}") for g in range(NKK)]
    for g in range(NKK):
        v3 = vt[g][:, 0:H * (HD + 1)].rearrange("p (h c) -> p h c", c=HD + 1)
        nc.vector.memset(v3[:, :, HD:HD + 1], 1.0)
        nc.vector.memset(vt[g][:, H * (HD + 1):VW], 0.0)
    # Q, zero-padded per head: head h occupies partitions 64*(h%2)..+64 of
    # qz[h], the other 64 partitions are zero -> scores matmul can use the
    # full 128-partition K^T pair tile as stationary (K=128, stays warm).
    qt_pool = ctx.enter_context(tc.tile_pool(name="qt", bufs=1))
    qz = [qt_pool.tile([P, FREE], BF16, name=f"qz{h}") for h in range(H)]
    for h in range(H):
        off = HD * ((h + 1) % 2)
        nc.vector.memset(qz[h][off:off + HD, :], 0.0)

    # ---- K^T projection, full batch: out[e, s] ----
    load_w((2, 3))
    with tc.tile_pool(name="projk_ps", bufs=8, space="PSUM") as ps_pool:
        for t in range(ET):
            eb = 2 + t // 4
            co = P * (t % 4)
            pss = [ps_pool.tile([P, FREE], F32, name=f"psk{t}_{sch}",
                                tag="proj") for sch in range(4)]
            # d-loop outer: 4 consecutive matmuls share one stationary tile
            for d in range(DCH):
                for sch in range(4):
                    nc.tensor.matmul(pss[sch][:],
                                     wblk[(eb, d)][:, co:co + P],
                                     xt[d][:, FREE * sch:FREE * (sch + 1)],
                                     start=(d == 0), stop=(d == DCH - 1))
            for sch in range(4):
                # cast + K-bias (per-partition) fused on ACT
                nc.scalar.activation(kt[t][:, FREE * sch:FREE * (sch + 1)],
                                     pss[sch][:], AF.Identity,
                                     bias=bqk_sb[:, 8 + t:9 + t])

    # ---- V projection, full batch, natural: out[s, e] ----
    load_w((4, 5))
    with tc.tile_pool(name="projv_ps", bufs=8, space="PSUM") as ps_pool:
        for st in range(NKK):
            pss = [ps_pool.tile([P, FREE], F32, name=f"psv{st}_{eb}",
                                tag="proj") for eb in range(2)]
            for d in range(DCH):
                for eb in range(2):
                    nc.tensor.matmul(pss[eb][:],
                                     xt[d][:, P * st:P * (st + 1)],
                                     wblk[(4 + eb, d)][:],
                                     start=(d == 0), stop=(d == DCH - 1))
            for eb in range(2):
                # cast to the [V|1] attention layout on DVE (no bias:
                # V-bias is folded into the output bias on host)
                v3 = vt[st][:, 0:H * (HD + 1)].rearrange(
                    "p (h c) -> p h c", c=HD + 1)
                nc.vector.tensor_copy(
                    v3[:, 8 * eb:8 * (eb + 1), 0:HD],
                    pss[eb].rearrange("p (h d) -> p h d", d=HD))

    # ---- Q projection (own 512 rows): out[e, q] ----
    load_w((0, 1))
    with tc.tile_pool(name="projq_ps", bufs=4, space="PSUM") as ps_pool:
        for t in range(ET):
            eb = t // 4
            co = P * (t % 4)
            ps = ps_pool.tile([P, FREE], F32, name=f"psq{t}", tag="proj")
            for d in range(DCH):
                nc.tensor.matmul(ps[:], wblk[(eb, d)][:, co:co + P], xq[d][:],
                                 start=(d == 0), stop=(d == DCH - 1))
            nc.scalar.activation(qz[2 * t][0:HD, :], ps[0:HD, :], AF.Identity,
                                 bias=bqk_sb[0:HD, t:t + 1])
            nc.scalar.activation(qz[2 * t + 1][HD:P, :], ps[HD:P, :],
                                 AF.Identity, bias=bqk_sb[HD:P, t:t + 1])

    # ---- prefetch output-projection weights ----
    wo_pool = ctx.enter_context(tc.tile_pool(name="wo", bufs=1))
    wo = []
    for p_ in range(DCH):
        t = wo_pool.tile([P, D], BF16, name=f"wo{p_}")
        nc.sync.dma_start(t[:], woutT.ap()[P * p_:P * (p_ + 1), :])
        wo.append(t)

    # ---- attention ----
    attn_sb_pool = ctx.enter_context(tc.tile_pool(name="attnsb", bufs=1))
    small_pool = ctx.enter_context(tc.tile_pool(name="small", bufs=4))
    attn_sb = [attn_sb_pool.tile([P, FREE], BF16, name=f"attnsb{p_}")
               for p_ in range(H // 2)]
    GRP = 2  # kk-chunks per score-psum tile
    with tc.tile_pool(name="sc_ps", bufs=3, space="PSUM") as sc_ps, \
         tc.tile_pool(name="at_ps", bufs=2, space="PSUM") as at_ps, \
         tc.tile_pool(name="e_sb", bufs=4) as e_pool:
        pending = []
        pv_pending = []

        def normalize(h, at):
            # attn = attnT[0:64] * (1/Z) ; Z = attnT row 64.
            # Broadcast Z first (PE outer-product waits only on the cheap
            # Z-row copy), then reciprocal on DVE off the PE critical path.
            koff = HD * (h % 2)
            zr = small_pool.tile([1, FREE], BF16, name=f"zr{h}", tag="zr")
            nc.vector.tensor_copy(zr[:], at[HD:HD + 1, :])
            bc = sc_ps.tile([HD, FREE], F32, name=f"bc{h}", tag="sc")
            nc.tensor.matmul(bc[:], ones_bf[:, :HD], zr[:],
                             start=True, stop=True)
            rzb = small_pool.tile([HD, FREE], F32, name=f"rzb{h}", tag="rzb")
            nc.vector.reciprocal(rzb[:], bc[:])
            nc.vector.tensor_mul(attn_sb[h // 2][koff:koff + HD, :],
                                 at[0:HD, :], rzb[:])

        def attn_v(h, at, g, e):
            for j in range(GRP):
                kk = GRP * g + j
                nc.tensor.matmul(at[:], vt[kk][:, 65 * h:65 * h + P],
                                 e[:, FREE * j:FREE * (j + 1)],
                                 start=(kk == 0), stop=(kk == NKK - 1))

        for h in range(H):
            ktile = h // 2
            q_rhs = qz[h][:]
            at = at_ps.tile([P, FREE], F32, name=f"at{h}", tag="at")
            for g in range(NKK // GRP):
                sc = sc_ps.tile([P, GRP * FREE], F32, name=f"sc{h}_{g}",
                                tag="sc")
                for j in range(GRP):
                    kk = GRP * g + j
                    nc.tensor.matmul(
                        sc[:, FREE * j:FREE * (j + 1)],
                        kt[ktile][:, P * kk:P * (kk + 1)],
                        q_rhs, start=True, stop=True)
                e = e_pool.tile([P, GRP * FREE], BF16, name=f"e{h}_{g}",
                                tag="e")
                nc.scalar.activation(e[:], sc[:], AF.Exp)
                # run the PV matmuls one group behind the scores stream, so
                # the PE never waits on the exp it just produced
                if pv_pending:
                    attn_v(*pv_pending.pop())
                pv_pending.append((h, at, g, e))
                if g == 2 and pending:
                    normalize(*pending.pop())
            pending.append((h, at))
        attn_v(*pv_pending.pop())
        normalize(*pending.pop())

    # ---- output projection ----
    with tc.tile_pool(name="out_ps", bufs=1, space="PSUM") as out_ps, \
         tc.tile_pool(name="out_sb", bufs=2) as out_sb_pool:
        ops = [[out_ps.tile([P, FREE], F32, name=f"op{st}_{eb}")
                for eb in range(2)] for st in range(SL // P)]
        for p_ in range(DCH):
            for st in range(SL // P):
                for eb in range(2):
                    nc.tensor.matmul(ops[st][eb],
                                     attn_sb[p_][:, P * st:P * (st + 1)],
                                     wo[p_][:, FREE * eb:FREE * (eb + 1)],
                                     start=(p_ == 0), stop=False)
        for st in range(SL // P):
            for eb in range(2):
                nc.tensor.matmul(ops[st][eb], ones_bf[:, :P],
                                 bout_sb[:, FREE * eb:FREE * (eb + 1)],
                                 start=False, stop=True)
            osb = out_sb_pool.tile([P, D], F32, name=f"osb{st}", tag="osb")
            for eb in range(2):
                nc.vector.tensor_copy(osb[:, FREE * eb:FREE * (eb + 1)],
                                      ops[st][eb])
            nc.sync.dma_start(out.ap()[P * st:P * (st + 1), :], osb[:])

    ctx.close()


_CACHE = {}


def _get_program():
    if "nc" not in _CACHE:
        _CACHE["nc"] = build_program()
    return _CACHE["nc"]


def prep_inputs(input_tensor, qkv_weight, qkv_bias, out_weight, out_bias):
    """Host-side shard + transpose + cast. Returns in_maps for 8 cores."""
    x = np.asarray(input_tensor, np.float32)
    wqkv = np.asarray(qkv_weight, np.float32).copy()
    bq = np.asarray(qkv_bias, np.float32).copy()
    wout = np.asarray(out_weight, np.float32)
    scale = 1.0 / np.sqrt(np.float32(HD))
    wqkv[:D] *= scale
    bq[:D] *= scale
    bf = ml_dtypes.bfloat16
    wqkvT = np.ascontiguousarray(wqkv.T).astype(bf)
    # Q/K biases, column-major per 128-feature tile: bqk[:, t] = bias tile t
    bqk = np.ascontiguousarray(bq[:2 * D].reshape(16, P).T).astype(bf)
    woutT = np.ascontiguousarray(wout.T).astype(bf)
    # V bias folded into output bias: probs @ (V + b_v) = probs @ V + b_v
    bout_eff = np.asarray(out_bias, np.float32) + wout @ bq[2 * D:]
    bout = bout_eff.reshape(1, D).astype(bf)
    xTb = [np.ascontiguousarray(x[b].T).astype(bf) for b in range(B)]
    in_maps = []
    for c in range(N_CORES):
        b, j = c // R, c % R
        xqT = np.ascontiguousarray(xTb[b][:, SL * j:SL * (j + 1)])
        in_maps.append({"xT": xTb[b], "xqT": xqT, "wqkvT": wqkvT,
                        "bqk": bqk, "woutT": woutT, "bout": bout})
    return in_maps


def kernel(input_tensor, qkv_weight, qkv_bias, out_weight, out_bias,
           **run_kwargs):
    nc = _get_program()
    in_maps = prep_inputs(input_tensor, qkv_weight, qkv_bias, out_weight,
                          out_bias)
    res = run_bass_kernel_spmd(nc, in_maps, core_ids=list(range(N_CORES)),
                               **run_kwargs)
    full = np.empty((B, S, D), np.float32)
    for c in range(N_CORES):
        b, j = c // R, c % R
        full[b, SL * j:SL * (j + 1), :] = res.results[c]["out"]
    if run_kwargs:
        kernel.last_results = res
    return full


# revision 25
# speedup vs baseline: 1.6357x; 1.0793x over previous
"""Multi-head attention (B=2, S=2048, D=1024, H=16) on 8 Trainium2 NeuronCores.

Sharding: core c handles (batch b=c//4, query chunk j=c%4 of 512 rows).
 - Each core computes K^T / V for its WHOLE batch locally (weights replicated,
   pre-transposed + bf16-cast on host; softmax scale folded into W_q) — no
   collectives, the PE stays continuously busy so the HAM clock-gate stays
   warm.
 - Q projected for the core's own 512 rows only.
 - Attention (all 16 heads, 512 queries x 2048 keys):
   scoresT = K_h @ Q_h^T  ->  exp on ACT  ->  attnT = [V_h|1]^T @ E
   (ones column gives the softmax denominator Z in row 64 of attnT psum).
 - Q/K biases folded into the projection casts (ACT Identity per-partition
   bias); V bias folded into the output bias on host (sum(probs) == 1).
 - Output projection local per 512-row chunk; final output assembled on host.
"""

import numpy as np
import ml_dtypes

import concourse.bass as bass
import concourse.mybir as mybir
import concourse.tile as tile
from concourse import bacc
from concourse.bass_utils import run_bass_kernel_spmd

BF16 = mybir.dt.bfloat16
F32 = mybir.dt.float32
AF = mybir.ActivationFunctionType

B, S, D = 2, 2048, 1024
H, HD = 16, 64
N_CORES = 8
R = 4            # cores per batch
SL = S // R      # local query rows per core (512)
P = 128
DCH = D // P     # 8 d-chunks
NKK = S // P     # 16 key chunks
ET = D // P      # 8 feature tiles per projection
FREE = 512


def build_program():
    nc = bacc.Bacc("TRN2", target_bir_lowering=False, debug=False,
                   num_devices=N_CORES)

    xT = nc.dram_tensor("xT", [D, S], BF16, kind="ExternalInput")
    xqT = nc.dram_tensor("xqT", [D, SL], BF16, kind="ExternalInput")
    wqkvT = nc.dram_tensor("wqkvT", [D, 3 * D], BF16, kind="ExternalInput")
    bqk = nc.dram_tensor("bqk", [P, 16], BF16, kind="ExternalInput")
    woutT = nc.dram_tensor("woutT", [D, D], BF16, kind="ExternalInput")
    bout = nc.dram_tensor("bout", [1, D], BF16, kind="ExternalInput")
    out = nc.dram_tensor("out", [SL, D], F32, kind="ExternalOutput")

    with tile.TileContext(nc) as tc:
        _build(nc, tc, xT, xqT, wqkvT, bqk, woutT, bout, out)
    nc.compile()
    return nc


def _build(nc, tc, xT, xqT, wqkvT, bqk, woutT, bout, out):
    from contextlib import ExitStack

    ctx = ExitStack()
    consts = ctx.enter_context(tc.tile_pool(name="consts", bufs=1))

    # ---- constants ----
    ones_bf = consts.tile([1, FREE], BF16, name="ones_bf")
    nc.vector.memset(ones_bf[:], 1.0)
    bqk_sb = consts.tile([P, 16], BF16, name="bqk_sb")
    nc.sync.dma_start(bqk_sb[:], bqk.ap())
    bout_sb = consts.tile([1, D], BF16, name="bout_sb")
    nc.sync.dma_start(bout_sb[:], bout.ap())

    # ---- resident input tiles ----
    xt_pool = ctx.enter_context(tc.tile_pool(name="xt", bufs=1))
    xt = []
    for i in range(DCH):
        t = xt_pool.tile([P, S], BF16, name=f"xt{i}")
        nc.sync.dma_start(t[:], xT.ap()[P * i:P * (i + 1), :])
        xt.append(t)
    xq = []
    for i in range(DCH):
        t = xt_pool.tile([P, SL], BF16, name=f"xq{i}")
        nc.sync.dma_start(t[:], xqT.ap()[P * i:P * (i + 1), :])
        xq.append(t)

    # ---- weight stream (K, V blocks first, then Q) ----
    w_pool = ctx.enter_context(tc.tile_pool(name="wq", bufs=16))
    wblk = {}

    def load_w(ebs):
        for eb in ebs:
            for d in range(DCH):
                t = w_pool.tile([P, FREE], BF16, name=f"w{eb}_{d}", tag="w")
                nc.gpsimd.dma_start(t[:], wqkvT.ap()[P * d:P * (d + 1),
                                                     FREE * eb:FREE * (eb + 1)])
                wblk[(eb, d)] = t

    # V layout: per key-tile, 16 heads x (64 V-features + ones col) packed at
    # stride 65, plus 64 zero columns of tail pad so a 128-wide stationary
    # slice [65h : 65h+128] is always in bounds. Columns 65..127 of that
    # slice hit neighbor-head data; they only feed psum rows 65..127 which
    # are never read. Full 128x128 stationary keeps the PE clock-gate warm.
    VW = H * (HD + 1) + HD  # 1104
    kv_pool = ctx.enter_context(tc.tile_pool(name="kv", bufs=1))
    kt = [kv_pool.tile([P, S], BF16, name=f"kt{t}") for t in range(ET)]
    vt = [kv_pool.tile([P, VW], BF16, name=f"vt{g}") for g in range(NKK)]
    for g in range(NKK):
        v3 = vt[g][:, 0:H * (HD + 1)].rearrange("p (h c) -> p h c", c=HD + 1)
        nc.vector.memset(v3[:, :, HD:HD + 1], 1.0)
        nc.vector.memset(vt[g][:, H * (HD + 1):VW], 0.0)
    # Q, zero-padded per head: head h occupies partitions 64*(h%2)..+64 of
    # qz[h], the other 64 partitions are zero -> scores matmul can use the
    # full 128-partition K^T pair tile as stationary (K=128, stays warm).
    qt_pool = ctx.enter_context(tc.tile_pool(name="qt", bufs=1))
    qz = [qt_pool.tile([P, FREE], BF16, name=f"qz{h}") for h in range(H)]
    for h in range(H):
        off = HD * ((h + 1) % 2)
        nc.vector.memset(qz[h][off:off + HD, :], 0.0)

    # ---- K^T projection, full batch: out[e, s] ----
    load_w((2, 3))
    with tc.tile_pool(name="projk_ps", bufs=8, space="PSUM") as ps_pool:
        for t in range(ET):
            eb = 2 + t // 4
            co = P * (t % 4)
            pss = [ps_pool.tile([P, FREE], F32, name=f"psk{t}_{sch}",
                                tag="proj") for sch in range(4)]
            # d-loop outer: 4 consecutive matmuls share one stationary tile
            for d in range(DCH):
                for sch in range(4):
                    nc.tensor.matmul(pss[sch][:],
                                     wblk[(eb, d)][:, co:co + P],
                                     xt[d][:, FREE * sch:FREE * (sch + 1)],
                                     start=(d == 0), stop=(d == DCH - 1))
            for sch in range(4):
                # cast + K-bias (per-partition) fused on ACT
                nc.scalar.activation(kt[t][:, FREE * sch:FREE * (sch + 1)],
                                     pss[sch][:], AF.Identity,
                                     bias=bqk_sb[:, 8 + t:9 + t])

    # ---- V projection, full batch, natural: out[s, e] ----
    load_w((4, 5))
    with tc.tile_pool(name="projv_ps", bufs=8, space="PSUM") as ps_pool:
        for st in range(NKK):
            pss = [ps_pool.tile([P, FREE], F32, name=f"psv{st}_{eb}",
                                tag="proj") for eb in range(2)]
            for d in range(DCH):
                for eb in range(2):
                    nc.tensor.matmul(pss[eb][:],
                                     xt[d][:, P * st:P * (st + 1)],
                                     wblk[(4 + eb, d)][:],
                                     start=(d == 0), stop=(d == DCH - 1))
            for eb in range(2):
                # cast to the [V|1] attention layout on DVE (no bias:
                # V-bias is folded into the output bias on host)
                v3 = vt[st][:, 0:H * (HD + 1)].rearrange(
                    "p (h c) -> p h c", c=HD + 1)
                nc.vector.tensor_copy(
                    v3[:, 8 * eb:8 * (eb + 1), 0:HD],
                    pss[eb].rearrange("p (h d) -> p h d", d=HD))

    # ---- Q projection (own 512 rows): out[e, q] ----
    load_w((0, 1))
    with tc.tile_pool(name="projq_ps", bufs=4, space="PSUM") as ps_pool:
        for t in range(ET):
            eb = t // 4
            co = P * (t % 4)
            ps = ps_pool.tile([P, FREE], F32, name=f"psq{t}", tag="proj")
            for d in range(DCH):
                nc.tensor.matmul(ps[:], wblk[(eb, d)][:, co:co + P], xq[d][:],
                                 start=(d == 0), stop=(d == DCH - 1))
            nc.scalar.activation(qz[2 * t][0:HD, :], ps[0:HD, :], AF.Identity,
                                 bias=bqk_sb[0:HD, t:t + 1])
            nc.scalar.activation(qz[2 * t + 1][HD:P, :], ps[HD:P, :],
                                 AF.Identity, bias=bqk_sb[HD:P, t:t + 1])

    # ---- prefetch output-projection weights ----
    wo_pool = ctx.enter_context(tc.tile_pool(name="wo", bufs=1))
    wo = []
    for p_ in range(DCH):
        t = wo_pool.tile([P, D], BF16, name=f"wo{p_}")
        nc.sync.dma_start(t[:], woutT.ap()[P * p_:P * (p_ + 1), :])
        wo.append(t)

    # ---- attention ----
    attn_sb_pool = ctx.enter_context(tc.tile_pool(name="attnsb", bufs=1))
    small_pool = ctx.enter_context(tc.tile_pool(name="small", bufs=4))
    attn_sb = [attn_sb_pool.tile([P, FREE], BF16, name=f"attnsb{p_}")
               for p_ in range(H // 2)]
    GRP = 2  # kk-chunks per score-psum tile
    with tc.tile_pool(name="sc_ps", bufs=2, space="PSUM") as sc_ps, \
         tc.tile_pool(name="at_ps", bufs=2, space="PSUM") as at_ps, \
         tc.tile_pool(name="bc_ps", bufs=2, space="PSUM") as bc_ps, \
         tc.tile_pool(name="e_sb", bufs=4) as e_pool:
        pending = []
        pv_pending = []

        def normalize(h, at):
            # attn = attnT[0:64] * (1/Z) ; Z = attnT row 64.
            # Broadcast Z first (PE outer-product waits only on the cheap
            # Z-row copy), then reciprocal on DVE off the PE critical path.
            koff = HD * (h % 2)
            zr = small_pool.tile([1, FREE], BF16, name=f"zr{h}", tag="zr")
            nc.vector.tensor_copy(zr[:], at[HD:HD + 1, :])
            bc = bc_ps.tile([HD, FREE], F32, name=f"bc{h}", tag="bc")
            nc.tensor.matmul(bc[:], ones_bf[:, :HD], zr[:],
                             start=True, stop=True)
            rzb = small_pool.tile([HD, FREE], F32, name=f"rzb{h}", tag="rzb")
            nc.vector.reciprocal(rzb[:], bc[:])
            nc.vector.tensor_mul(attn_sb[h // 2][koff:koff + HD, :],
                                 at[0:HD, :], rzb[:])

        def attn_v(h, at, g, e):
            for j in range(GRP):
                kk = GRP * g + j
                nc.tensor.matmul(at[:], vt[kk][:, 65 * h:65 * h + P],
                                 e[:, FREE * j:FREE * (j + 1)],
                                 start=(kk == 0), stop=(kk == NKK - 1))

        for h in range(H):
            ktile = h // 2
            q_rhs = qz[h][:]
            at = at_ps.tile([P, FREE], F32, name=f"at{h}", tag="at")
            for g in range(NKK // GRP):
                sc = sc_ps.tile([P, GRP * FREE], F32, name=f"sc{h}_{g}",
                                tag="sc")
                for j in range(GRP):
                    kk = GRP * g + j
                    nc.tensor.matmul(
                        sc[:, FREE * j:FREE * (j + 1)],
                        kt[ktile][:, P * kk:P * (kk + 1)],
                        q_rhs, start=True, stop=True)
                e = e_pool.tile([P, GRP * FREE], BF16, name=f"e{h}_{g}",
                                tag="e")
                nc.scalar.activation(e[:], sc[:], AF.Exp)
                # run the PV matmuls one group behind the scores stream, so
                # the PE never waits on the exp it just produced
                if pv_pending:
                    attn_v(*pv_pending.pop())
                pv_pending.append((h, at, g, e))
                if g == 2 and pending:
                    normalize(*pending.pop())
            pending.append((h, at))
        attn_v(*pv_pending.pop())
        normalize(*pending.pop())

    # ---- output projection ----
    with tc.tile_pool(name="out_ps", bufs=1, space="PSUM") as out_ps, \
         tc.tile_pool(name="out_sb", bufs=2) as out_sb_pool:
        ops = [[out_ps.tile([P, FREE], F32, name=f"op{st}_{eb}")
                for eb in range(2)] for st in range(SL // P)]
        for p_ in range(DCH):
            for st in range(SL // P):
                for eb in range(2):
                    nc.tensor.matmul(ops[st][eb],
                                     attn_sb[p_][:, P * st:P * (st + 1)],
                                     wo[p_][:, FREE * eb:FREE * (eb + 1)],
                                     start=(p_ == 0), stop=False)
        for st in range(SL // P):
            for eb in range(2):
                nc.tensor.matmul(ops[st][eb], ones_bf[:, :P],
                                 bout_sb[:, FREE * eb:FREE * (eb + 1)],
                                 start=False, stop=True)
            osb = out_sb_pool.tile([P, D], F32, name=f"osb{st}", tag="osb")
            for eb in range(2):
                nc.vector.tensor_copy(osb[:, FREE * eb:FREE * (eb + 1)],
                                      ops[st][eb])
            nc.sync.dma_start(out.ap()[P * st:P * (st + 1), :], osb[:])

    ctx.close()


_CACHE = {}


def _get_program():
    if "nc" not in _CACHE:
        _CACHE["nc"] = build_program()
    return _CACHE["nc"]


def prep_inputs(input_tensor, qkv_weight, qkv_bias, out_weight, out_bias):
    """Host-side shard + transpose + cast. Returns in_maps for 8 cores."""
    x = np.asarray(input_tensor, np.float32)
    wqkv = np.asarray(qkv_weight, np.float32).copy()
    bq = np.asarray(qkv_bias, np.float32).copy()
    wout = np.asarray(out_weight, np.float32)
    scale = 1.0 / np.sqrt(np.float32(HD))
    wqkv[:D] *= scale
    bq[:D] *= scale
    bf = ml_dtypes.bfloat16
    wqkvT = np.ascontiguousarray(wqkv.T).astype(bf)
    # Q/K biases, column-major per 128-feature tile: bqk[:, t] = bias tile t
    bqk = np.ascontiguousarray(bq[:2 * D].reshape(16, P).T).astype(bf)
    woutT = np.ascontiguousarray(wout.T).astype(bf)
    # V bias folded into output bias: probs @ (V + b_v) = probs @ V + b_v
    bout_eff = np.asarray(out_bias, np.float32) + wout @ bq[2 * D:]
    bout = bout_eff.reshape(1, D).astype(bf)
    xTb = [np.ascontiguousarray(x[b].T).astype(bf) for b in range(B)]
    in_maps = []
    for c in range(N_CORES):
        b, j = c // R, c % R
        xqT = np.ascontiguousarray(xTb[b][:, SL * j:SL * (j + 1)])
        in_maps.append({"xT": xTb[b], "xqT": xqT, "wqkvT": wqkvT,
                        "bqk": bqk, "woutT": woutT, "bout": bout})
    return in_maps


def kernel(input_tensor, qkv_weight, qkv_bias, out_weight, out_bias,
           **run_kwargs):
    nc = _get_program()
    in_maps = prep_inputs(input_tensor, qkv_weight, qkv_bias, out_weight,
                          out_bias)
    res = run_bass_kernel_spmd(nc, in_maps, core_ids=list(range(N_CORES)),
                               **run_kwargs)
    full = np.empty((B, S, D), np.float32)
    for c in range(N_CORES):
        b, j = c // R, c % R
        full[b, SL * j:SL * (j + 1), :] = res.results[c]["out"]
    if run_kwargs:
        kernel.last_results = res
    return full
